# revision 5
# baseline (speedup 1.0000x reference)
"""CPMAnt attention kernel for 8 TRN2 NeuronCores.

Sharding: 8 cores = 2 batches x 4 head-groups (4 heads each).
Each core computes its batch's QKV projections for its 4 heads, attention
with position bias, and a row-parallel partial of the output projection.
Host sums the 4 partials per batch (Megatron row-parallel reduce done on
host at gather time; no collectives needed).

Matmuls run in bf16 with f32 PSUM accumulation, except the Q/K projections
which run fp8-e4m3 DoubleRow (2 contraction chunks per matmul): the CPMAnt
scores (std ~4e-4 after scaling) are tiny against the position bias
(std ~1), so fp8 noise on Q/K is invisible in the output. Weights are
pre-scaled by 64 on the host to sit in fp8's normal range. KT/QT are also
STORED in fp8 (scores again), which halves their SBUF footprint and lets
the score matmuls run fp8xfp8 (same PE rate as bf16). V/attention/output-
projection stay bf16 (their error hits the output linearly).

Softmax denominator via column-tiled strip matmuls: all 4 heads of an
s-block keep their E = exp(S)*exp(pb) tiles alive; Z_h = ones^T E_h runs
as four concurrent M=32 column strips of the PE array (tile_position via
sliced PSUM base partitions), so the 4 heads' denominators cost ~one
matmul stream instead of four. Each head's reciprocal row is then
broadcast to 128 partitions with a tiny K=32 ones-matmul, and the
normalize multiplies OT (copied to SBUF bf16 per head) by it.

Transposed-operand formulation (no on-device transposes):
  KT8[o,t]  = wk8.T @ hk8      (fp8 DoubleRow, stored fp8)
  V  [t,o]  = hkvT.T @ wvT     (bf16)
  QT8[o,s]  = wq8.T @ hq8      (fp8 DoubleRow, stored fp8)
  ST [t,s]  = KT8_h.T @ QT8_h  (fp8 operands, bf16-rate)
  ET        = exp(ST*ES) * exp(pbT)    (ACT exp over chunk pairs, DVE mult)
  OT[o,s]  += V_h.T @ ET       -> OT_sb bf16
  Zq[32h,s] = ones32.T @ ET    (4 concurrent column strips, one PSUM bank)
  rz        = recip(Zq); rz128_h = ones.T @ rz_h (K=32 broadcast matmul)
  AT        = OT_sb * rz128
  out[s,m] += AT_h.T @ woT

DMA queue split: all input loads go through the Sync HWDGE ring (pure
prefetch FIFO), all output stores through GpSimd SWDGE, so stores waiting
on compute never head-of-line-block the next block's prefetches.
"""

import math
import os

import numpy as np
import ml_dtypes

import concourse.bass as bass
import concourse.bacc as bacc
import concourse.tile as tile
from concourse import mybir
from concourse.bass_utils import run_bass_kernel_spmd

BF16 = ml_dtypes.bfloat16
FP8 = mybir.dt.np(mybir.dt.float8e4)

# Problem shapes (hardcoded per contest contract).
B, LQ, LK = 2, 2048, 2048
DM, H, DH = 2048, 16, 128
P = 128            # partitions
NCORES = 8
HPC = 4            # heads per core
OC = HPC * DH      # 512 output-proj contraction per core
DC = DM // P       # 16 d-chunks
TC = LK // P       # 16 t-chunks
SB = 4             # s-blocks per 2048
NB = LQ // SB      # 512
NPAIR = TC // 2    # 8 score-chunk pairs per block

W8SCALE = 64.0     # host pre-scale for fp8 weights
KV_SCALE = 1.0 / math.sqrt(DM)
OUT_SCALE = 1.0 / math.sqrt(H * DH)
# KT8 = K_A * k_true, QT8 = Q_A * q_true / sqrt(DH); exp undoes K_A*Q_A.
K_A = 64.0
Q_A = 512.0
K8_COPY = K_A / (W8SCALE * math.sqrt(DM))
Q8_COPY = Q_A / (W8SCALE * math.sqrt(DM) * math.sqrt(DH))
EXP_SCALE = 1.0 / (K_A * Q_A)

_PROGRAM = None          # cached compiled Bass program
_LAST_RESULTS = None     # BassKernelResults from the most recent run


def build_program():
    f32 = mybir.dt.float32
    bf16 = mybir.dt.bfloat16
    f8 = mybir.dt.float8e4
    DR = mybir.MatmulPerfMode.DoubleRow
    nc = bacc.Bacc()

    # Streamed tensors are stored block-major so every DMA slice is fully
    # contiguous (8-16KB per-partition lines -> full HBM rate).
    hq8 = nc.dram_tensor("hq8", [SB, P, DC, NB], f8, kind="ExternalInput")
    hk8 = nc.dram_tensor("hk8", [SB, P, DC, NB], f8, kind="ExternalInput")
    hkv = nc.dram_tensor("hkv", [SB, P, DC, NB], bf16, kind="ExternalInput")
    wq8 = nc.dram_tensor("wq8", [P, DC, OC], f8, kind="ExternalInput")
    wk8 = nc.dram_tensor("wk8", [P, DC, OC], f8, kind="ExternalInput")
    wvT = nc.dram_tensor("wvT", [P, DC, OC], bf16, kind="ExternalInput")
    woT = nc.dram_tensor("woT", [P, HPC, DM], bf16, kind="ExternalInput")
    pbe = nc.dram_tensor("pbe", [HPC, SB, P, TC, NB], bf16, kind="ExternalInput")
    out = nc.dram_tensor("out", [P, LQ // P, DM], f32, kind="ExternalOutput")

    Copy = mybir.ActivationFunctionType.Copy
    Exp = mybir.ActivationFunctionType.Exp
    Mult = mybir.AluOpType.mult

    with tile.TileContext(nc) as tc:
        with (
            tc.tile_pool(name="persist", bufs=1) as persist,
            tc.tile_pool(name="kv", bufs=1) as kvp,
            tc.tile_pool(name="hq_s", bufs=2) as hqs,
        ):
            KT8 = kvp.tile([P, HPC, LK], f8)
            V = kvp.tile([P, TC, OC], bf16)

            def emit_hq_dma(j):
                # on the ACT HWDGE ring so pb loads on the Sync ring can
                # never head-of-line-block the next QT projection
                hq_sl = hqs.tile([P, DC, NB], f8, tag="hq", name="hq_sl")
                nc.scalar.dma_start(out=hq_sl, in_=hq8[j])
                return hq_sl

            # ---- KT / V projections (hidden_kv) ----
            with (
                tc.tile_pool(name="wkv", bufs=1) as wkvp,
                tc.tile_pool(name="h8s", bufs=3) as h8s,
                tc.tile_pool(name="hstream", bufs=2) as hs,
                tc.tile_pool(name="psA", bufs=6, space="PSUM") as psA,
            ):
                # Warmup matmuls: fill the cold-start DMA wait with junk PE
                # work so HAM unthrottles before the real stream begins.
                warm = persist.tile([P, P], bf16, name="warm")
                nc.vector.memset(warm, 0.0)
                wps = psA.tile([P, P], f32, tag="psA")
                for i in range(128):
                    nc.tensor.matmul(
                        wps, lhsT=warm, rhs=warm,
                        start=(i == 0), stop=(i == 127),
                    )

                # K projections first: only 2MB of fp8 (wk8 + first slice) is
                # startup-critical; V's bf16 loads trail behind on the ring.
                wk_sb = wkvp.tile([P, DC, OC], f8)
                nc.sync.dma_start(out=wk_sb, in_=wk8[:])
                k_sl0 = h8s.tile([P, DC, NB], f8, tag="h8")
                nc.sync.dma_start(out=k_sl0, in_=hk8[0])
                ones_sb = persist.tile([P, P], bf16)
                nc.vector.memset(ones_sb, 1.0)
                wq_sb = persist.tile([P, DC, OC], f8)
                woT_sb = persist.tile([P, HPC, DM], bf16)
                hq_tiles = []

                for j in range(SB):
                    if j == 0:
                        k_sl = k_sl0
                    else:
                        k_sl = h8s.tile([P, DC, NB], f8, tag="h8")
                        nc.sync.dma_start(out=k_sl, in_=hk8[j])
                    for h in range(HPC):
                        ps = psA.tile([P, NB], f32, tag="psA")
                        for d in range(0, DC, 2):
                            nc.tensor.matmul(
                                ps,
                                lhsT=wk_sb[:, d:d + 2, h * P:(h + 1) * P],
                                rhs=k_sl[:, d:d + 2, :],
                                start=(d == 0),
                                stop=(d == DC - 2),
                                perf_mode=DR,
                            )
                        nc.scalar.activation(
                            KT8[:, h, j * NB:(j + 1) * NB], ps, Copy, scale=K8_COPY
                        )
                    if j == 0:
                        # Non-startup-critical loads go on the ACT HWDGE
                        # ring, emitted behind j0's KT copies so they don't
                        # steal HBM bandwidth from the first hidden slices.
                        hq_tiles += [emit_hq_dma(0), emit_hq_dma(1)]
                        nc.scalar.dma_start(out=wq_sb, in_=wq8[:])
                        nc.scalar.dma_start(out=woT_sb, in_=woT[:])

                wv_sb = wkvp.tile([P, DC, OC], bf16)
                nc.sync.dma_start(out=wv_sb, in_=wvT[:])
                for j in range(SB):
                    h_sl = hs.tile([P, DC, NB], bf16, tag="h")
                    nc.sync.dma_start(out=h_sl, in_=hkv[j])
                    for t4 in range(4):
                        ps = psA.tile([P, NB], f32, tag="psA")
                        for d in range(DC):
                            nc.tensor.matmul(
                                ps,
                                lhsT=h_sl[:, d, t4 * P:(t4 + 1) * P],
                                rhs=wv_sb[:, d, :],
                                start=(d == 0),
                                stop=(d == DC - 1),
                            )
                        nc.scalar.activation(
                            V[:, j * 4 + t4, :], ps, Copy, scale=KV_SCALE
                        )

            # ---- fused main loop over s-blocks ----
            with (
                tc.tile_pool(name="qt", bufs=2) as qtp,
                tc.tile_pool(name="at", bufs=1) as atp,
                tc.tile_pool(name="ot", bufs=1) as otp,
                tc.tile_pool(name="pb", bufs=3) as pbp,
                tc.tile_pool(name="es", bufs=2) as esp,
                tc.tile_pool(name="E", bufs=4) as Ep,
                tc.tile_pool(name="rz", bufs=1) as rzp,
                tc.tile_pool(name="rzb", bufs=2) as rzbp,
                tc.tile_pool(name="cst", bufs=2) as csp,
                tc.tile_pool(name="psS", bufs=2, space="PSUM") as psS,
                tc.tile_pool(name="psO", bufs=2, space="PSUM") as psO,
                tc.tile_pool(name="psZ", bufs=2, space="PSUM") as psZ,
            ):
                def emit_qt_proj(hq_sl):
                    QT8j = qtp.tile([P, HPC, NB], f8, tag="qt", name="QT8j")
                    for hp in range(HPC // 2):
                        ps = psS.tile([P, 2 * NB], f32, tag="big", name="psq")
                        for h2 in range(2):
                            h = 2 * hp + h2
                            for d in range(0, DC, 2):
                                nc.tensor.matmul(
                                    ps[:, h2 * NB:(h2 + 1) * NB],
                                    lhsT=wq_sb[:, d:d + 2, h * P:(h + 1) * P],
                                    rhs=hq_sl[:, d:d + 2, :],
                                    start=(d == 0),
                                    stop=(d == DC - 2),
                                    perf_mode=DR,
                                    skip_group_check=True,
                                )
                        nc.vector.tensor_scalar_mul(
                            QT8j[:, 2 * hp:2 * hp + 2, :],
                            ps.rearrange("p (c n) -> p c n", c=2),
                            Q8_COPY,
                        )
                    return QT8j

                # Rolling position-bias prefetch, 2 blocks deep.
                blocks = [(j, h) for j in range(SB) for h in range(HPC)]

                def emit_pb_dma(j, h):
                    pb_sl = pbp.tile([P, TC, NB], bf16, tag="pb", name="pb_sl")
                    nc.sync.dma_start(out=pb_sl, in_=pbe[h, j])
                    return pb_sl

                pb_tiles = {bl: emit_pb_dma(*bl) for bl in blocks[:2]}

                QT8j = emit_qt_proj(hq_tiles.pop(0))
                hq_next = hq_tiles.pop(0)
                for j in range(SB):
                    OT_sb = otp.tile([P, HPC, NB], bf16, tag="ot")
                    E_tiles = []
                    for h in range(HPC):
                        pb_sl = pb_tiles.pop((j, h))
                        ahead = blocks.index((j, h)) + 2
                        if ahead < len(blocks):
                            pb_tiles[blocks[ahead]] = emit_pb_dma(*blocks[ahead])
                        E_sl = Ep.tile([P, TC, NB], bf16, tag="E")
                        E_tiles.append(E_sl)
                        O_ps = psO.tile([P, NB], f32, tag="psO")

                        def av(t):
                            nc.tensor.matmul(
                                O_ps,
                                lhsT=V[:, t, h * DH:(h + 1) * DH],
                                rhs=E_sl[:, t, :],
                                start=(t == 0),
                                stop=(t == TC - 1),
                                skip_group_check=True,
                            )

                        for p in range(NPAIR):
                            S_ps = psS.tile([P, 2 * NB], f32, tag="big")
                            for q in range(2):
                                nc.tensor.matmul(
                                    S_ps[:, q * NB:(q + 1) * NB],
                                    lhsT=KT8[:, h, (2 * p + q) * P:(2 * p + q + 1) * P],
                                    rhs=QT8j[:, h, :],
                                    start=True,
                                    stop=True,
                                    skip_group_check=True,
                                )
                            eS = esp.tile([P, 2 * NB], bf16, tag="es")
                            nc.scalar.activation(eS, S_ps, Exp, scale=EXP_SCALE)
                            nc.vector.tensor_tensor(
                                E_sl[:, 2 * p:2 * p + 2, :],
                                eS.rearrange("p (c n) -> p c n", c=2),
                                pb_sl[:, 2 * p:2 * p + 2, :],
                                Mult,
                            )
                            if p >= 2:
                                av(2 * p - 4)
                                av(2 * p - 3)
                        for t in range(TC - 4, TC):
                            av(t)
                        # Free the PSUM bank for the next head; the end-of-
                        # block normalize reads the bf16 SBUF copy instead.
                        nc.scalar.activation(OT_sb[:, h, :], O_ps, Copy)

                    # Softmax denominators for all 4 heads as concurrent
                    # 32-wide column strips: one matmul stream's worth of PE
                    # time instead of four.
                    Zq_ps = psZ.tile([P, NB], f32, tag="psZ")
                    for t in range(TC):
                        for h in range(HPC):
                            nc.tensor.matmul(
                                Zq_ps[32 * h:32 * (h + 1), :],
                                lhsT=ones_sb[:, 0:32],
                                rhs=E_tiles[h][:, t, :],
                                start=(t == 0),
                                stop=(t == TC - 1),
                                skip_group_check=True,
                                tile_position=(0, 32 * h),
                            )

                    # Next s-block's QT projection goes here: it has no
                    # dependency on this block's attention tail, so it fills
                    # the PE bubble while DVE computes the reciprocal.
                    if j < SB - 1:
                        QT8next = emit_qt_proj(hq_next)
                        if j < SB - 2:
                            hq_next = emit_hq_dma(j + 2)
                        elif j == SB - 2:
                            hq_next = None

                    rz_f32 = rzp.tile([P, NB], f32, tag="rz")
                    nc.vector.reciprocal_approx_fast(rz_f32, Zq_ps)
                    rz_bf = rzp.tile([P, NB], bf16, tag="rzc")
                    nc.vector.tensor_copy(rz_bf, rz_f32)

                    ATj = atp.tile([P, HPC, NB], bf16, tag="at")
                    for h in range(HPC):
                        # Broadcast strip h's reciprocal to all 128
                        # partitions with a K=32 ones-matmul (sums the 32
                        # identical rows -> fold the 1/32 into the copy).
                        rz_ps = psZ.tile([P, NB], f32, tag="psZ")
                        nc.tensor.matmul(
                            rz_ps,
                            lhsT=ones_sb[32 * h:32 * (h + 1), :],
                            rhs=rz_bf[32 * h:32 * (h + 1), :],
                            start=True,
                            stop=True,
                            tile_position=(32 * h, 0),
                        )
                        rz128 = rzbp.tile([P, NB], bf16, tag="rzb")
                        nc.scalar.activation(rz128, rz_ps, Copy, scale=1.0 / 32.0)
                        nc.vector.tensor_tensor(
                            ATj[:, h, :], OT_sb[:, h, :], rz128, Mult
                        )

                    # out-projection for this s-block (row-parallel partial)
                    for sc4 in range(NB // P):
                        sc = j * (NB // P) + sc4
                        for mbp in range(DM // NB // 2):
                            ps = psS.tile([P, 2 * NB], f32, tag="big")
                            for mb2 in range(2):
                                mb = 2 * mbp + mb2
                                for oc in range(HPC):
                                    nc.tensor.matmul(
                                        ps[:, mb2 * NB:(mb2 + 1) * NB],
                                        lhsT=ATj[:, oc, sc4 * P:(sc4 + 1) * P],
                                        rhs=woT_sb[:, oc, mb * NB:(mb + 1) * NB],
                                        start=(oc == 0),
                                        stop=(oc == HPC - 1),
                                        skip_group_check=True,
                                    )
                            cst = csp.tile([P, 2 * NB], f32, tag="cs")
                            nc.vector.tensor_scalar_mul(cst, ps, OUT_SCALE)
                            nc.gpsimd.dma_start(
                                out=out[:, sc, mbp * 2 * NB:(mbp + 1) * 2 * NB],
                                in_=cst,
                            )

                    if j < SB - 1:
                        QT8j = QT8next

    nc.compile()
    return nc


def _get_program():
    global _PROGRAM
    if _PROGRAM is None:
        _PROGRAM = build_program()
    return _PROGRAM


def make_in_maps(hidden_q, hidden_kv, attention_mask, position_bias, wq, wk, wv, wo):
    """Host-side shard + transpose + cast for all 8 cores."""
    f32 = np.float32

    def dxp(x):  # [n, (dc p)] -> [p, dc, n]  (transpose with d on partitions)
        n = x.shape[0]
        return np.ascontiguousarray(x.reshape(n, DC, P).transpose(2, 1, 0))

    def blocked(t):  # [p, dc, n] -> [SB, p, dc, NB]  (contiguous DMA slices)
        return np.ascontiguousarray(
            t.reshape(P, DC, SB, NB).transpose(2, 0, 1, 3)
        )

    hq8_b = [blocked(dxp(np.asarray(hidden_q[b], f32))).astype(FP8) for b in range(B)]
    hkv_t = [blocked(dxp(np.asarray(hidden_kv[b], f32))) for b in range(B)]
    hk8_b = [t.astype(FP8) for t in hkv_t]
    hkv_b = [t.astype(BF16) for t in hkv_t]

    mask = np.asarray(attention_mask)
    mask_all_ones = bool(mask.all())

    w_by_hg = []
    for hg in range(HPC):
        rows = slice(hg * OC, (hg + 1) * OC)
        wq8 = (dxp(np.asarray(wq[rows], f32)) * W8SCALE).astype(FP8)
        wk8 = (dxp(np.asarray(wk[rows], f32)) * W8SCALE).astype(FP8)
        wvT = dxp(np.asarray(wv[rows], f32)).astype(BF16)
        woT = np.ascontiguousarray(
            np.asarray(wo[:, rows], f32).reshape(DM, HPC, P).transpose(2, 1, 0)
        ).astype(BF16)
        w_by_hg.append((wq8, wk8, wvT, woT))

    in_maps = []
    for core in range(NCORES):
        b, hg = divmod(core, HPC)
        pb_sel = np.asarray(position_bias[hg * HPC:(hg + 1) * HPC], f32)
        pbT = pb_sel.reshape(HPC, LQ, TC, P).transpose(0, 3, 2, 1)  # [h,p,tc,s]
        pbe = np.exp(pbT, dtype=f32)
        if not mask_all_ones:
            # mask folded multiplicatively into exp(pb): zeroed keys drop out
            # of both the numerator and the softmax denominator, matching
            # where(mask, score, -inf) + where(mask, probs, 0).
            mT = mask[b].T.reshape(TC, P, LQ).transpose(1, 0, 2)
            pbe = pbe * mT[None].astype(f32)
        # block-major on s: [h, p, tc, s] -> [h, SB, p, tc, NB]
        pbe = np.ascontiguousarray(
            pbe.reshape(HPC, P, TC, SB, NB).transpose(0, 3, 1, 2, 4)
        )
        wq8, wk8, wvT, woT = w_by_hg[hg]
        in_maps.append(
            {
                "hq8": hq8_b[b],
                "hk8": hk8_b[b],
                "hkv": hkv_b[b],
                "wq8": wq8,
                "wk8": wk8,
                "wvT": wvT,
                "woT": woT,
                "pbe": pbe.astype(BF16),
            }
        )
    return in_maps


def gather_output(results):
    """Sum the 4 row-parallel partials per batch; un-permute to [B, LQ, DM]."""
    out = np.zeros((B, LQ, DM), np.float32)
    for core in range(NCORES):
        b = core // HPC
        part = results[core]["out"]  # [P, LQ//P, DM]
        out[b] += part.transpose(1, 0, 2).reshape(LQ, DM)
    return out


def kernel(hidden_q, hidden_kv, attention_mask, position_bias, wq, wk, wv, wo):
    global _LAST_RESULTS
    nc = _get_program()
    in_maps = make_in_maps(
        hidden_q, hidden_kv, attention_mask, position_bias, wq, wk, wv, wo
    )
    trace = os.environ.get("KERNEL_TRACE", "0") == "1"
    res = run_bass_kernel_spmd(
        nc,
        in_maps,
        core_ids=list(range(NCORES)),
        trace=trace,
        trace_cores=[0] if trace else None,
    )
    _LAST_RESULTS = res
    return gather_output(res.results)


# revision 8
# speedup vs baseline: 1.0395x; 1.0395x over previous
"""CPMAnt attention kernel for 8 TRN2 NeuronCores.

Sharding: 8 cores = 2 batches x 4 head-groups (4 heads each).
Each core computes its batch's QKV projections for its 4 heads, attention
with position bias, and a row-parallel partial of the output projection.
Host sums the 4 partials per batch (Megatron row-parallel reduce done on
host at gather time; no collectives needed).

Matmuls run in bf16 with f32 PSUM accumulation, except the Q/K projections
which run fp8-e4m3 DoubleRow (2 contraction chunks per matmul): the CPMAnt
scores (std ~4e-4 after scaling) are tiny against the position bias
(std ~1), so fp8 noise on Q/K is invisible in the output. Weights are
pre-scaled by 64 on the host to sit in fp8's normal range. KT/QT are also
STORED in fp8 (scores again), which halves their SBUF footprint and lets
the score matmuls run fp8xfp8 (same PE rate as bf16). V/attention/output-
projection stay bf16 (their error hits the output linearly).

Softmax denominator via column-tiled strip matmuls: all 4 heads of an
s-block keep their E = exp(S)*exp(pb) tiles alive; Z_h = ones^T E_h runs
as four concurrent M=32 column strips of the PE array (tile_position via
sliced PSUM base partitions), so the 4 heads' denominators cost ~one
matmul stream instead of four. Each head's reciprocal row is then
broadcast to 128 partitions with a tiny K=32 ones-matmul, and the
normalize multiplies OT (copied to SBUF bf16 per head) by it.

Transposed-operand formulation (no on-device transposes):
  KT8[o,t]  = wk8.T @ hk8      (fp8 DoubleRow, stored fp8)
  V  [t,o]  = hkvT.T @ wvT     (bf16)
  QT8[o,s]  = wq8.T @ hq8      (fp8 DoubleRow, stored fp8)
  ST [t,s]  = KT8_h.T @ QT8_h  (fp8 operands, bf16-rate)
  ET        = exp(ST*ES) * exp(pbT)    (ACT exp over chunk pairs, DVE mult)
  OT[o,s]  += V_h.T @ ET       -> OT_sb bf16
  Zq[32h,s] = ones32.T @ ET    (4 concurrent column strips, one PSUM bank)
  rz        = recip(Zq); rz128_h = ones.T @ rz_h (K=32 broadcast matmul)
  AT        = OT_sb * rz128
  out[s,m] += AT_h.T @ woT

DMA queue split: all input loads go through the Sync HWDGE ring (pure
prefetch FIFO), all output stores through GpSimd SWDGE, so stores waiting
on compute never head-of-line-block the next block's prefetches.
"""

import math
import os

import numpy as np
import ml_dtypes

import concourse.bass as bass
import concourse.bacc as bacc
import concourse.tile as tile
from concourse import mybir
from concourse.bass_utils import run_bass_kernel_spmd

BF16 = ml_dtypes.bfloat16
FP8 = mybir.dt.np(mybir.dt.float8e4)

# Problem shapes (hardcoded per contest contract).
B, LQ, LK = 2, 2048, 2048
DM, H, DH = 2048, 16, 128
P = 128            # partitions
NCORES = 8
HPC = 4            # heads per core
OC = HPC * DH      # 512 output-proj contraction per core
DC = DM // P       # 16 d-chunks
TC = LK // P       # 16 t-chunks
SB = 4             # s-blocks per 2048
NB = LQ // SB      # 512
NPAIR = TC // 2    # 8 score-chunk pairs per block

W8SCALE = 64.0     # host pre-scale for fp8 weights
KV_SCALE = 1.0 / math.sqrt(DM)
OUT_SCALE = 1.0 / math.sqrt(H * DH)
# KT8 = K_A * k_true, QT8 = Q_A * q_true / sqrt(DH); exp undoes K_A*Q_A.
K_A = 64.0
Q_A = 512.0
K8_COPY = K_A / (W8SCALE * math.sqrt(DM))
Q8_COPY = Q_A / (W8SCALE * math.sqrt(DM) * math.sqrt(DH))
EXP_SCALE = 1.0 / (K_A * Q_A)

_PROGRAM = None          # cached compiled Bass program
_LAST_RESULTS = None     # BassKernelResults from the most recent run


def build_program():
    f32 = mybir.dt.float32
    bf16 = mybir.dt.bfloat16
    f8 = mybir.dt.float8e4
    DR = mybir.MatmulPerfMode.DoubleRow
    nc = bacc.Bacc()

    # Streamed tensors are stored block-major so every DMA slice is fully
    # contiguous (8-16KB per-partition lines -> full HBM rate).
    hq8 = nc.dram_tensor("hq8", [SB, P, DC, NB], f8, kind="ExternalInput")
    hk8 = nc.dram_tensor("hk8", [SB, P, DC, NB], f8, kind="ExternalInput")
    hkv = nc.dram_tensor("hkv", [SB, P, DC, NB], bf16, kind="ExternalInput")
    wq8 = nc.dram_tensor("wq8", [P, DC, OC], f8, kind="ExternalInput")
    wk8 = nc.dram_tensor("wk8", [P, DC, OC], f8, kind="ExternalInput")
    wvT = nc.dram_tensor("wvT", [P, DC, OC], bf16, kind="ExternalInput")
    woT = nc.dram_tensor("woT", [P, HPC, DM], bf16, kind="ExternalInput")
    pbe = nc.dram_tensor("pbe", [HPC, SB, P, TC, NB], bf16, kind="ExternalInput")
    out = nc.dram_tensor("out", [P, LQ // P, DM], f32, kind="ExternalOutput")

    Copy = mybir.ActivationFunctionType.Copy
    Exp = mybir.ActivationFunctionType.Exp
    Mult = mybir.AluOpType.mult

    with tile.TileContext(nc) as tc:
        with (
            tc.tile_pool(name="persist", bufs=1) as persist,
            tc.tile_pool(name="kv", bufs=1) as kvp,
            tc.tile_pool(name="hq_s", bufs=2) as hqs,
        ):
            KT8 = kvp.tile([P, HPC, LK], f8)
            V = kvp.tile([P, TC, OC], bf16)

            def emit_hq_dma(j):
                # on the ACT HWDGE ring so pb loads on the Sync ring can
                # never head-of-line-block the next QT projection
                hq_sl = hqs.tile([P, DC, NB], f8, tag="hq", name="hq_sl")
                nc.scalar.dma_start(out=hq_sl, in_=hq8[j])
                return hq_sl

            # ---- KT / V projections (hidden_kv) ----
            with (
                tc.tile_pool(name="wkv", bufs=1) as wkvp,
                tc.tile_pool(name="h8s", bufs=3) as h8s,
                tc.tile_pool(name="hstream", bufs=2) as hs,
                tc.tile_pool(name="psA", bufs=6, space="PSUM") as psA,
            ):
                # Warmup matmuls: fill the cold-start DMA wait with junk PE
                # work so HAM unthrottles before the real stream begins.
                warm = persist.tile([P, P], bf16, name="warm")
                nc.vector.memset(warm, 0.0)
                wps = psA.tile([P, P], f32, tag="psA")
                for i in range(128):
                    nc.tensor.matmul(
                        wps, lhsT=warm, rhs=warm,
                        start=(i == 0), stop=(i == 127),
                    )

                # K projections first: only 2MB of fp8 (wk8 + first slice) is
                # startup-critical; V's bf16 loads trail behind on the ring.
                wk_sb = wkvp.tile([P, DC, OC], f8)
                nc.sync.dma_start(out=wk_sb, in_=wk8[:])
                k_sl0 = h8s.tile([P, DC, NB], f8, tag="h8")
                nc.sync.dma_start(out=k_sl0, in_=hk8[0])
                ones_sb = persist.tile([P, P], bf16)
                nc.vector.memset(ones_sb, 1.0)
                wq_sb = persist.tile([P, DC, OC], f8)
                woT_sb = persist.tile([P, HPC, DM], bf16)
                hq_tiles = []

                for j in range(SB):
                    if j == 0:
                        k_sl = k_sl0
                    else:
                        k_sl = h8s.tile([P, DC, NB], f8, tag="h8")
                        nc.sync.dma_start(out=k_sl, in_=hk8[j])
                    for h in range(HPC):
                        ps = psA.tile([P, NB], f32, tag="psA")
                        for d in range(0, DC, 2):
                            nc.tensor.matmul(
                                ps,
                                lhsT=wk_sb[:, d:d + 2, h * P:(h + 1) * P],
                                rhs=k_sl[:, d:d + 2, :],
                                start=(d == 0),
                                stop=(d == DC - 2),
                                perf_mode=DR,
                            )
                        nc.scalar.activation(
                            KT8[:, h, j * NB:(j + 1) * NB], ps, Copy, scale=K8_COPY
                        )
                    if j == 0:
                        # Non-startup-critical loads go on the ACT HWDGE
                        # ring, emitted behind j0's KT copies so they don't
                        # steal HBM bandwidth from the first hidden slices.
                        hq_tiles += [emit_hq_dma(0), emit_hq_dma(1)]
                        nc.scalar.dma_start(out=wq_sb, in_=wq8[:])
                        nc.scalar.dma_start(out=woT_sb, in_=woT[:])

                wv_sb = wkvp.tile([P, DC, OC], bf16)
                nc.sync.dma_start(out=wv_sb, in_=wvT[:])
                for j in range(SB):
                    h_sl = hs.tile([P, DC, NB], bf16, tag="h")
                    nc.sync.dma_start(out=h_sl, in_=hkv[j])
                    for t4 in range(4):
                        ps = psA.tile([P, NB], f32, tag="psA")
                        for d in range(DC):
                            nc.tensor.matmul(
                                ps,
                                lhsT=h_sl[:, d, t4 * P:(t4 + 1) * P],
                                rhs=wv_sb[:, d, :],
                                start=(d == 0),
                                stop=(d == DC - 1),
                            )
                        nc.scalar.activation(
                            V[:, j * 4 + t4, :], ps, Copy, scale=KV_SCALE
                        )

            # ---- fused main loop over s-blocks ----
            with (
                tc.tile_pool(name="qt", bufs=2) as qtp,
                tc.tile_pool(name="at", bufs=1) as atp,
                tc.tile_pool(name="ot", bufs=1) as otp,
                tc.tile_pool(name="pb", bufs=3) as pbp,
                tc.tile_pool(name="es", bufs=2) as esp,
                tc.tile_pool(name="E", bufs=4) as Ep,
                tc.tile_pool(name="rz", bufs=1) as rzp,
                tc.tile_pool(name="rzb", bufs=2) as rzbp,
                tc.tile_pool(name="cst", bufs=2) as csp,
                tc.tile_pool(name="psS", bufs=2, space="PSUM") as psS,
                tc.tile_pool(name="psOP", bufs=1, space="PSUM") as psOP,
                tc.tile_pool(name="psO", bufs=1, space="PSUM") as psO,
                tc.tile_pool(name="psZ", bufs=1, space="PSUM") as psZ,
            ):
                # PE work-stealing queue: single-matmul thunks of dependency-
                # free deferred work (previous block's out-projection, next
                # block's QT projection) that are woven between the S/AV
                # matmuls so PE never idles while ACT works through the exps.
                stolen = []

                def steal(n):
                    for _ in range(min(n, len(stolen))):
                        stolen.pop(0)()

                def queue_qt_proj(hq_sl):
                    """Queue the next block's QT projection; returns the
                    (not-yet-written) fp8 QT tile."""
                    QT8n = qtp.tile([P, HPC, NB], f8, tag="qt", name="QT8n")
                    for hp in range(HPC // 2):
                        ps = psOP.tile([P, 2 * NB], f32, tag="op", name="psq")
                        for h2 in range(2):
                            h = 2 * hp + h2
                            for d in range(0, DC, 2):
                                def mm(h=h, d=d, ps=ps, h2=h2):
                                    nc.tensor.matmul(
                                        ps[:, h2 * NB:(h2 + 1) * NB],
                                        lhsT=wq_sb[:, d:d + 2, h * P:(h + 1) * P],
                                        rhs=hq_sl[:, d:d + 2, :],
                                        start=(d == 0),
                                        stop=(d == DC - 2),
                                        perf_mode=DR,
                                        skip_group_check=True,
                                    )
                                stolen.append(mm)
                        prev = stolen.pop()

                        def last_mm(prev=prev, hp=hp, ps=ps):
                            prev()
                            nc.vector.tensor_scalar_mul(
                                QT8n[:, 2 * hp:2 * hp + 2, :],
                                ps.rearrange("p (c n) -> p c n", c=2),
                                Q8_COPY,
                            )
                        stolen.append(last_mm)
                    return QT8n

                def queue_outproj(ATj, j):
                    """Queue block j's out-projection (row-parallel partial)."""
                    for sc4 in range(NB // P):
                        sc = j * (NB // P) + sc4
                        for mbp in range(DM // NB // 2):
                            ps = psOP.tile([P, 2 * NB], f32, tag="op", name="psop")
                            for mb2 in range(2):
                                mb = 2 * mbp + mb2
                                for oc in range(HPC):
                                    def mm(ps=ps, mb2=mb2, mb=mb, oc=oc,
                                           ATj=ATj, sc4=sc4):
                                        nc.tensor.matmul(
                                            ps[:, mb2 * NB:(mb2 + 1) * NB],
                                            lhsT=ATj[:, oc, sc4 * P:(sc4 + 1) * P],
                                            rhs=woT_sb[:, oc, mb * NB:(mb + 1) * NB],
                                            start=(oc == 0),
                                            stop=(oc == HPC - 1),
                                            skip_group_check=True,
                                        )
                                    stolen.append(mm)
                            def store(ps=ps, sc=sc, mbp=mbp):
                                cst = csp.tile([P, 2 * NB], f32, tag="cs")
                                nc.vector.tensor_scalar_mul(cst, ps, OUT_SCALE)
                                nc.gpsimd.dma_start(
                                    out=out[:, sc, mbp * 2 * NB:(mbp + 1) * 2 * NB],
                                    in_=cst,
                                )
                            prev = stolen.pop()
                            def last_mm(prev=prev, store=store):
                                prev()
                                store()
                            stolen.append(last_mm)

                # Rolling position-bias prefetch, 2 blocks deep.
                blocks = [(j, h) for j in range(SB) for h in range(HPC)]

                def emit_pb_dma(j, h):
                    pb_sl = pbp.tile([P, TC, NB], bf16, tag="pb", name="pb_sl")
                    nc.sync.dma_start(out=pb_sl, in_=pbe[h, j])
                    return pb_sl

                pb_tiles = {bl: emit_pb_dma(*bl) for bl in blocks[:2]}

                # j=0's QT projection runs immediately (nothing to overlap).
                QT8j = queue_qt_proj(hq_tiles.pop(0))
                steal(len(stolen))
                hq_next = hq_tiles.pop(0)
                AT_prev = None
                for j in range(SB):
                    # Deferred work woven into this block's S/AV stream.
                    if AT_prev is not None:
                        queue_outproj(AT_prev, j - 1)
                    if j < SB - 1:
                        QT8next = queue_qt_proj(hq_next)
                        if j < SB - 2:
                            hq_next = emit_hq_dma(j + 2)
                        elif j == SB - 2:
                            hq_next = None

                    OT_sb = otp.tile([P, HPC, NB], bf16, tag="ot")
                    E_tiles = []
                    for h in range(HPC):
                        pb_sl = pb_tiles.pop((j, h))
                        ahead = blocks.index((j, h)) + 2
                        if ahead < len(blocks):
                            pb_tiles[blocks[ahead]] = emit_pb_dma(*blocks[ahead])
                        E_sl = Ep.tile([P, TC, NB], bf16, tag="E")
                        E_tiles.append(E_sl)
                        O_ps = psO.tile([P, NB], f32, tag="psO")

                        def av(t):
                            nc.tensor.matmul(
                                O_ps,
                                lhsT=V[:, t, h * DH:(h + 1) * DH],
                                rhs=E_sl[:, t, :],
                                start=(t == 0),
                                stop=(t == TC - 1),
                                skip_group_check=True,
                            )

                        for p in range(NPAIR):
                            S_ps = psS.tile([P, 2 * NB], f32, tag="big")
                            for q in range(2):
                                nc.tensor.matmul(
                                    S_ps[:, q * NB:(q + 1) * NB],
                                    lhsT=KT8[:, h, (2 * p + q) * P:(2 * p + q + 1) * P],
                                    rhs=QT8j[:, h, :],
                                    start=True,
                                    stop=True,
                                    skip_group_check=True,
                                )
                            steal(1 if p >= 2 else 2)
                            eS = esp.tile([P, 2 * NB], bf16, tag="es")
                            nc.scalar.activation(eS, S_ps, Exp, scale=EXP_SCALE)
                            nc.vector.tensor_tensor(
                                E_sl[:, 2 * p:2 * p + 2, :],
                                eS.rearrange("p (c n) -> p c n", c=2),
                                pb_sl[:, 2 * p:2 * p + 2, :],
                                Mult,
                            )
                            if p >= 2:
                                av(2 * p - 4)
                                av(2 * p - 3)
                        for t in range(TC - 4, TC):
                            av(t)
                            steal(1)
                        # Free the PSUM bank for the next head; the end-of-
                        # block normalize reads the bf16 SBUF copy instead.
                        nc.vector.tensor_copy(OT_sb[:, h, :], O_ps)

                    # Softmax denominators for all 4 heads as concurrent
                    # 32-wide column strips: one matmul stream's worth of PE
                    # time instead of four.
                    Zq_ps = psZ.tile([P, NB], f32, tag="psZ")
                    for t in range(TC):
                        for h in range(HPC):
                            nc.tensor.matmul(
                                Zq_ps[32 * h:32 * (h + 1), :],
                                lhsT=ones_sb[:, 0:32],
                                rhs=E_tiles[h][:, t, :],
                                start=(t == 0),
                                stop=(t == TC - 1),
                                skip_group_check=True,
                                tile_position=(0, 32 * h),
                            )

                    # Drain any leftover deferred matmuls: they fill the PE
                    # while DVE computes the reciprocal chain below.
                    steal(len(stolen))

                    rz_f32 = rzp.tile([P, NB], f32, tag="rz")
                    nc.vector.reciprocal_approx_fast(rz_f32, Zq_ps)
                    rz_bf = rzp.tile([P, NB], bf16, tag="rzc")
                    nc.vector.tensor_copy(rz_bf, rz_f32)

                    ATj = atp.tile([P, HPC, NB], bf16, tag="at")
                    for h in range(HPC):
                        # Broadcast strip h's reciprocal to all 128
                        # partitions with a K=32 ones-matmul (sums the 32
                        # identical rows -> fold the 1/32 into the copy).
                        rz_ps = psZ.tile([P, NB], f32, tag="psZ")
                        nc.tensor.matmul(
                            rz_ps,
                            lhsT=ones_sb[32 * h:32 * (h + 1), :],
                            rhs=rz_bf[32 * h:32 * (h + 1), :],
                            start=True,
                            stop=True,
                            tile_position=(32 * h, 0),
                        )
                        rz128 = rzbp.tile([P, NB], bf16, tag="rzb")
                        nc.scalar.activation(rz128, rz_ps, Copy, scale=1.0 / 32.0)
                        nc.vector.tensor_tensor(
                            ATj[:, h, :], OT_sb[:, h, :], rz128, Mult
                        )
                    AT_prev = ATj
                    if j < SB - 1:
                        QT8j = QT8next

                # Final block's out-projection has nothing left to hide
                # behind; emit it directly.
                queue_outproj(AT_prev, SB - 1)
                steal(len(stolen))

    nc.compile()
    return nc


def _get_program():
    global _PROGRAM
    if _PROGRAM is None:
        _PROGRAM = build_program()
    return _PROGRAM


def make_in_maps(hidden_q, hidden_kv, attention_mask, position_bias, wq, wk, wv, wo):
    """Host-side shard + transpose + cast for all 8 cores."""
    f32 = np.float32

    def dxp(x):  # [n, (dc p)] -> [p, dc, n]  (transpose with d on partitions)
        n = x.shape[0]
        return np.ascontiguousarray(x.reshape(n, DC, P).transpose(2, 1, 0))

    def blocked(t):  # [p, dc, n] -> [SB, p, dc, NB]  (contiguous DMA slices)
        return np.ascontiguousarray(
            t.reshape(P, DC, SB, NB).transpose(2, 0, 1, 3)
        )

    hq8_b = [blocked(dxp(np.asarray(hidden_q[b], f32))).astype(FP8) for b in range(B)]
    hkv_t = [blocked(dxp(np.asarray(hidden_kv[b], f32))) for b in range(B)]
    hk8_b = [t.astype(FP8) for t in hkv_t]
    hkv_b = [t.astype(BF16) for t in hkv_t]

    mask = np.asarray(attention_mask)
    mask_all_ones = bool(mask.all())

    w_by_hg = []
    for hg in range(HPC):
        rows = slice(hg * OC, (hg + 1) * OC)
        wq8 = (dxp(np.asarray(wq[rows], f32)) * W8SCALE).astype(FP8)
        wk8 = (dxp(np.asarray(wk[rows], f32)) * W8SCALE).astype(FP8)
        wvT = dxp(np.asarray(wv[rows], f32)).astype(BF16)
        woT = np.ascontiguousarray(
            np.asarray(wo[:, rows], f32).reshape(DM, HPC, P).transpose(2, 1, 0)
        ).astype(BF16)
        w_by_hg.append((wq8, wk8, wvT, woT))

    in_maps = []
    for core in range(NCORES):
        b, hg = divmod(core, HPC)
        pb_sel = np.asarray(position_bias[hg * HPC:(hg + 1) * HPC], f32)
        pbT = pb_sel.reshape(HPC, LQ, TC, P).transpose(0, 3, 2, 1)  # [h,p,tc,s]
        pbe = np.exp(pbT, dtype=f32)
        if not mask_all_ones:
            # mask folded multiplicatively into exp(pb): zeroed keys drop out
            # of both the numerator and the softmax denominator, matching
            # where(mask, score, -inf) + where(mask, probs, 0).
            mT = mask[b].T.reshape(TC, P, LQ).transpose(1, 0, 2)
            pbe = pbe * mT[None].astype(f32)
        # block-major on s: [h, p, tc, s] -> [h, SB, p, tc, NB]
        pbe = np.ascontiguousarray(
            pbe.reshape(HPC, P, TC, SB, NB).transpose(0, 3, 1, 2, 4)
        )
        wq8, wk8, wvT, woT = w_by_hg[hg]
        in_maps.append(
            {
                "hq8": hq8_b[b],
                "hk8": hk8_b[b],
                "hkv": hkv_b[b],
                "wq8": wq8,
                "wk8": wk8,
                "wvT": wvT,
                "woT": woT,
                "pbe": pbe.astype(BF16),
            }
        )
    return in_maps


def gather_output(results):
    """Sum the 4 row-parallel partials per batch; un-permute to [B, LQ, DM]."""
    out = np.zeros((B, LQ, DM), np.float32)
    for core in range(NCORES):
        b = core // HPC
        part = results[core]["out"]  # [P, LQ//P, DM]
        out[b] += part.transpose(1, 0, 2).reshape(LQ, DM)
    return out


def kernel(hidden_q, hidden_kv, attention_mask, position_bias, wq, wk, wv, wo):
    global _LAST_RESULTS
    nc = _get_program()
    in_maps = make_in_maps(
        hidden_q, hidden_kv, attention_mask, position_bias, wq, wk, wv, wo
    )
    trace = os.environ.get("KERNEL_TRACE", "0") == "1"
    res = run_bass_kernel_spmd(
        nc,
        in_maps,
        core_ids=list(range(NCORES)),
        trace=trace,
        trace_cores=[0] if trace else None,
    )
    _LAST_RESULTS = res
    return gather_output(res.results)


# revision 11
# speedup vs baseline: 1.0518x; 1.0118x over previous
"""CPMAnt attention kernel for 8 TRN2 NeuronCores.

Sharding: 8 cores = 2 batches x 4 head-groups (4 heads each).
Each core computes its batch's QKV projections for its 4 heads, attention
with position bias, and a row-parallel partial of the output projection.
Host sums the 4 partials per batch (Megatron row-parallel reduce done on
host at gather time; no collectives needed).

Matmuls run in bf16 with f32 PSUM accumulation, except the Q/K projections
which run fp8-e4m3 DoubleRow (2 contraction chunks per matmul): the CPMAnt
scores (std ~4e-4 after scaling) are tiny against the position bias
(std ~1), so fp8 noise on Q/K is invisible in the output. Weights are
pre-scaled by 64 on the host to sit in fp8's normal range. KT/QT are also
STORED in fp8 (scores again), which halves their SBUF footprint and lets
the score matmuls run fp8xfp8 (same PE rate as bf16). V/attention/output-
projection stay bf16 (their error hits the output linearly).

Softmax denominator via column-tiled strip matmuls: all 4 heads of an
s-block keep their E = exp(S)*exp(pb) tiles alive; Z_h = ones^T E_h runs
as four concurrent M=32 column strips of the PE array (tile_position via
sliced PSUM base partitions), so the 4 heads' denominators cost ~one
matmul stream instead of four. Each head's reciprocal row is then
broadcast to 128 partitions with a tiny K=32 ones-matmul, and the
normalize multiplies OT (copied to SBUF bf16 per head) by it.

Transposed-operand formulation (no on-device transposes):
  KT8[o,t]  = wk8.T @ hk8      (fp8 DoubleRow, stored fp8)
  V  [t,o]  = hkvT.T @ wvT     (bf16)
  QT8[o,s]  = wq8.T @ hq8      (fp8 DoubleRow, stored fp8)
  ST [t,s]  = KT8_h.T @ QT8_h  (fp8 operands, bf16-rate)
  ET        = exp(ST*ES) * exp(pbT)    (ACT exp over chunk pairs, DVE mult)
  OT[o,s]  += V_h.T @ ET       -> OT_sb bf16
  Zq[32h,s] = ones32.T @ ET    (4 concurrent column strips, one PSUM bank)
  rz        = recip(Zq); rz128_h = ones.T @ rz_h (K=32 broadcast matmul)
  AT        = OT_sb * rz128
  out[s,m] += AT_h.T @ woT

DMA queue split: all input loads go through the Sync HWDGE ring (pure
prefetch FIFO), all output stores through GpSimd SWDGE, so stores waiting
on compute never head-of-line-block the next block's prefetches.
"""

import math
import os

import numpy as np
import ml_dtypes

import concourse.bass as bass
import concourse.bacc as bacc
import concourse.tile as tile
from concourse import mybir
from concourse.bass_utils import run_bass_kernel_spmd

BF16 = ml_dtypes.bfloat16
FP8 = mybir.dt.np(mybir.dt.float8e4)

# Problem shapes (hardcoded per contest contract).
B, LQ, LK = 2, 2048, 2048
DM, H, DH = 2048, 16, 128
P = 128            # partitions
NCORES = 8
HPC = 4            # heads per core
OC = HPC * DH      # 512 output-proj contraction per core
DC = DM // P       # 16 d-chunks
TC = LK // P       # 16 t-chunks
SB = 4             # s-blocks per 2048
NB = LQ // SB      # 512
NPAIR = TC // 2    # 8 score-chunk pairs per block

W8SCALE = 64.0     # host pre-scale for fp8 weights
KV_SCALE = 1.0 / math.sqrt(DM)
OUT_SCALE = 1.0 / math.sqrt(H * DH)
# KT8 = K_A * k_true, QT8 = Q_A * q_true / sqrt(DH); exp undoes K_A*Q_A.
K_A = 64.0
Q_A = 512.0
K8_COPY = K_A / (W8SCALE * math.sqrt(DM))
Q8_COPY = Q_A / (W8SCALE * math.sqrt(DM) * math.sqrt(DH))
EXP_SCALE = 1.0 / (K_A * Q_A)

_PROGRAM = None          # cached compiled Bass program
_LAST_RESULTS = None     # BassKernelResults from the most recent run


def build_program():
    f32 = mybir.dt.float32
    bf16 = mybir.dt.bfloat16
    f8 = mybir.dt.float8e4
    DR = mybir.MatmulPerfMode.DoubleRow
    nc = bacc.Bacc()

    # Streamed tensors are stored block-major so every DMA slice is fully
    # contiguous (8-16KB per-partition lines -> full HBM rate).
    hq8 = nc.dram_tensor("hq8", [SB, P, DC, NB], f8, kind="ExternalInput")
    hk8 = nc.dram_tensor("hk8", [SB, P, DC, NB], f8, kind="ExternalInput")
    hkv = nc.dram_tensor("hkv", [SB, P, DC, NB], bf16, kind="ExternalInput")
    wq8 = nc.dram_tensor("wq8", [P, DC, OC], f8, kind="ExternalInput")
    wk8 = nc.dram_tensor("wk8", [P, DC, OC], f8, kind="ExternalInput")
    wvT = nc.dram_tensor("wvT", [P, DC, OC], bf16, kind="ExternalInput")
    woT = nc.dram_tensor("woT", [P, HPC, DM], bf16, kind="ExternalInput")
    pbe = nc.dram_tensor("pbe", [HPC, SB, P, TC, NB], bf16, kind="ExternalInput")
    out = nc.dram_tensor("out", [P, LQ // P, DM], f32, kind="ExternalOutput")

    Copy = mybir.ActivationFunctionType.Copy
    Exp = mybir.ActivationFunctionType.Exp
    Mult = mybir.AluOpType.mult

    with tile.TileContext(nc) as tc:
        with (
            tc.tile_pool(name="persist", bufs=1) as persist,
            tc.tile_pool(name="kv", bufs=1) as kvp,
            tc.tile_pool(name="hq_s", bufs=2) as hqs,
        ):
            KT8 = kvp.tile([P, HPC, LK], f8)
            V = kvp.tile([P, TC, OC], bf16)

            def emit_hq_dma(j):
                # on the ACT HWDGE ring so pb loads on the Sync ring can
                # never head-of-line-block the next QT projection
                hq_sl = hqs.tile([P, DC, NB], f8, tag="hq", name="hq_sl")
                nc.scalar.dma_start(out=hq_sl, in_=hq8[j])
                return hq_sl

            # ---- KT / V projections (hidden_kv) ----
            with (
                tc.tile_pool(name="wkv", bufs=1) as wkvp,
                tc.tile_pool(name="h8s", bufs=3) as h8s,
                tc.tile_pool(name="hstream", bufs=2) as hs,
                tc.tile_pool(name="psA", bufs=6, space="PSUM") as psA,
            ):
                # Warmup matmuls: fill the cold-start DMA wait with junk PE
                # work so HAM unthrottles before the real stream begins.
                warm = persist.tile([P, P], bf16, name="warm")
                nc.vector.memset(warm, 0.0)
                wps = psA.tile([P, P], f32, tag="psA")
                NWARM = 168
                for i in range(NWARM):
                    nc.tensor.matmul(
                        wps, lhsT=warm, rhs=warm,
                        start=(i == 0), stop=(i == NWARM - 1),
                    )

                # K projections first: only 2MB of fp8 (wk8 + first slice) is
                # startup-critical; V's bf16 loads trail behind on the ring.
                wk_sb = wkvp.tile([P, DC, OC], f8)
                nc.sync.dma_start(out=wk_sb, in_=wk8[:])
                k_sl0 = h8s.tile([P, DC, NB], f8, tag="h8")
                nc.sync.dma_start(out=k_sl0, in_=hk8[0])
                ones_sb = persist.tile([P, P], bf16)
                nc.vector.memset(ones_sb, 1.0)
                wq_sb = persist.tile([P, DC, OC], f8)
                woT_sb = persist.tile([P, HPC, DM], bf16)
                hq_tiles = []

                for j in range(SB):
                    if j == 0:
                        k_sl = k_sl0
                    else:
                        k_sl = h8s.tile([P, DC, NB], f8, tag="h8")
                        nc.sync.dma_start(out=k_sl, in_=hk8[j])
                    for h in range(HPC):
                        ps = psA.tile([P, NB], f32, tag="psA")
                        for d in range(0, DC, 2):
                            nc.tensor.matmul(
                                ps,
                                lhsT=wk_sb[:, d:d + 2, h * P:(h + 1) * P],
                                rhs=k_sl[:, d:d + 2, :],
                                start=(d == 0),
                                stop=(d == DC - 2),
                                perf_mode=DR,
                            )
                        nc.scalar.activation(
                            KT8[:, h, j * NB:(j + 1) * NB], ps, Copy, scale=K8_COPY
                        )
                    if j == 0:
                        # Non-startup-critical loads go on the ACT HWDGE
                        # ring, emitted behind j0's KT copies so they don't
                        # steal HBM bandwidth from the first hidden slices.
                        hq_tiles += [emit_hq_dma(0), emit_hq_dma(1)]
                        nc.scalar.dma_start(out=wq_sb, in_=wq8[:])
                        nc.scalar.dma_start(out=woT_sb, in_=woT[:])

                wv_sb = wkvp.tile([P, DC, OC], bf16)
                nc.sync.dma_start(out=wv_sb, in_=wvT[:])
                for j in range(SB):
                    h_sl = hs.tile([P, DC, NB], bf16, tag="h")
                    nc.sync.dma_start(out=h_sl, in_=hkv[j])
                    for t4 in range(4):
                        ps = psA.tile([P, NB], f32, tag="psA")
                        for d in range(DC):
                            nc.tensor.matmul(
                                ps,
                                lhsT=h_sl[:, d, t4 * P:(t4 + 1) * P],
                                rhs=wv_sb[:, d, :],
                                start=(d == 0),
                                stop=(d == DC - 1),
                            )
                        nc.scalar.activation(
                            V[:, j * 4 + t4, :], ps, Copy, scale=KV_SCALE
                        )

            # ---- fused main loop over s-blocks ----
            with (
                tc.tile_pool(name="qt", bufs=2) as qtp,
                tc.tile_pool(name="at", bufs=1) as atp,
                tc.tile_pool(name="ot", bufs=1) as otp,
                tc.tile_pool(name="pb", bufs=3) as pbp,
                tc.tile_pool(name="es", bufs=2) as esp,
                tc.tile_pool(name="E", bufs=4) as Ep,
                tc.tile_pool(name="rz", bufs=1) as rzp,
                tc.tile_pool(name="cst", bufs=2) as csp,
                tc.tile_pool(name="psS", bufs=2, space="PSUM") as psS,
                tc.tile_pool(name="psOP", bufs=1, space="PSUM") as psOP,
                tc.tile_pool(name="psO", bufs=1, space="PSUM") as psO,
                tc.tile_pool(name="psZ", bufs=1, space="PSUM") as psZ,
            ):
                # PE work-stealing queue: single-matmul thunks of dependency-
                # free deferred work (previous block's out-projection, next
                # block's QT projection) that are woven between the S/AV
                # matmuls so PE never idles while ACT works through the exps.
                stolen = []

                def steal(n):
                    for _ in range(min(n, len(stolen))):
                        stolen.pop(0)()

                def queue_qt_proj(hq_sl):
                    """Queue the next block's QT projection; returns the
                    (not-yet-written) fp8 QT tile."""
                    QT8n = qtp.tile([P, HPC, NB], f8, tag="qt", name="QT8n")
                    for hp in range(HPC // 2):
                        ps = psOP.tile([P, 2 * NB], f32, tag="op", name="psq")
                        for h2 in range(2):
                            h = 2 * hp + h2
                            for d in range(0, DC, 2):
                                def mm(h=h, d=d, ps=ps, h2=h2):
                                    nc.tensor.matmul(
                                        ps[:, h2 * NB:(h2 + 1) * NB],
                                        lhsT=wq_sb[:, d:d + 2, h * P:(h + 1) * P],
                                        rhs=hq_sl[:, d:d + 2, :],
                                        start=(d == 0),
                                        stop=(d == DC - 2),
                                        perf_mode=DR,
                                        skip_group_check=True,
                                    )
                                stolen.append(mm)
                        prev = stolen.pop()

                        def last_mm(prev=prev, hp=hp, ps=ps):
                            prev()
                            nc.vector.tensor_scalar_mul(
                                QT8n[:, 2 * hp:2 * hp + 2, :],
                                ps.rearrange("p (c n) -> p c n", c=2),
                                Q8_COPY,
                            )
                        stolen.append(last_mm)
                    return QT8n

                def queue_outproj(ATj, j):
                    """Queue block j's out-projection (row-parallel partial)."""
                    for sc4 in range(NB // P):
                        sc = j * (NB // P) + sc4
                        for mbp in range(DM // NB // 2):
                            ps = psOP.tile([P, 2 * NB], f32, tag="op", name="psop")
                            for mb2 in range(2):
                                mb = 2 * mbp + mb2
                                for oc in range(HPC):
                                    def mm(ps=ps, mb2=mb2, mb=mb, oc=oc,
                                           ATj=ATj, sc4=sc4):
                                        nc.tensor.matmul(
                                            ps[:, mb2 * NB:(mb2 + 1) * NB],
                                            lhsT=ATj[:, oc, sc4 * P:(sc4 + 1) * P],
                                            rhs=woT_sb[:, oc, mb * NB:(mb + 1) * NB],
                                            start=(oc == 0),
                                            stop=(oc == HPC - 1),
                                            skip_group_check=True,
                                        )
                                    stolen.append(mm)
                            def store(ps=ps, sc=sc, mbp=mbp):
                                cst = csp.tile([P, 2 * NB], f32, tag="cs")
                                nc.vector.tensor_scalar_mul(cst, ps, OUT_SCALE)
                                nc.gpsimd.dma_start(
                                    out=out[:, sc, mbp * 2 * NB:(mbp + 1) * 2 * NB],
                                    in_=cst,
                                )
                            prev = stolen.pop()
                            def last_mm(prev=prev, store=store):
                                prev()
                                store()
                            stolen.append(last_mm)

                # Rolling position-bias prefetch, 2 blocks deep.
                blocks = [(j, h) for j in range(SB) for h in range(HPC)]

                def emit_pb_dma(j, h):
                    pb_sl = pbp.tile([P, TC, NB], bf16, tag="pb", name="pb_sl")
                    nc.sync.dma_start(out=pb_sl, in_=pbe[h, j])
                    return pb_sl

                pb_tiles = {bl: emit_pb_dma(*bl) for bl in blocks[:2]}

                # j=0's QT projection runs immediately (nothing to overlap).
                QT8j = queue_qt_proj(hq_tiles.pop(0))
                steal(len(stolen))
                hq_next = hq_tiles.pop(0)
                AT_prev = None
                for j in range(SB):
                    # Deferred work woven into this block's S/AV stream.
                    if AT_prev is not None:
                        queue_outproj(AT_prev, j - 1)
                    if j < SB - 1:
                        QT8next = queue_qt_proj(hq_next)
                        if j < SB - 2:
                            hq_next = emit_hq_dma(j + 2)
                        elif j == SB - 2:
                            hq_next = None

                    OT_sb = otp.tile([P, HPC, NB], bf16, tag="ot")
                    E_tiles = []
                    for h in range(HPC):
                        pb_sl = pb_tiles.pop((j, h))
                        ahead = blocks.index((j, h)) + 2
                        if ahead < len(blocks):
                            pb_tiles[blocks[ahead]] = emit_pb_dma(*blocks[ahead])
                        E_sl = Ep.tile([P, TC, NB], bf16, tag="E")
                        E_tiles.append(E_sl)
                        O_ps = psO.tile([P, NB], f32, tag="psO")

                        def av(t):
                            nc.tensor.matmul(
                                O_ps,
                                lhsT=V[:, t, h * DH:(h + 1) * DH],
                                rhs=E_sl[:, t, :],
                                start=(t == 0),
                                stop=(t == TC - 1),
                                skip_group_check=True,
                            )

                        for p in range(NPAIR):
                            S_ps = psS.tile([P, 2 * NB], f32, tag="big")
                            for q in range(2):
                                nc.tensor.matmul(
                                    S_ps[:, q * NB:(q + 1) * NB],
                                    lhsT=KT8[:, h, (2 * p + q) * P:(2 * p + q + 1) * P],
                                    rhs=QT8j[:, h, :],
                                    start=True,
                                    stop=True,
                                    skip_group_check=True,
                                )
                            steal(1 if p >= 2 else 2)
                            eS = esp.tile([P, 2 * NB], bf16, tag="es")
                            nc.scalar.activation(eS, S_ps, Exp, scale=EXP_SCALE)
                            nc.vector.tensor_tensor(
                                E_sl[:, 2 * p:2 * p + 2, :],
                                eS.rearrange("p (c n) -> p c n", c=2),
                                pb_sl[:, 2 * p:2 * p + 2, :],
                                Mult,
                            )
                            if p >= 2:
                                av(2 * p - 4)
                                av(2 * p - 3)
                        for t in range(TC - 4, TC):
                            av(t)
                            steal(1)
                        # Free the PSUM bank for the next head; the end-of-
                        # block normalize reads the bf16 SBUF copy instead.
                        nc.vector.tensor_copy(OT_sb[:, h, :], O_ps)

                    # Softmax denominators for all 4 heads as concurrent
                    # 32-wide column strips: one matmul stream's worth of PE
                    # time instead of four.
                    Zq_ps = psZ.tile([P, NB], f32, tag="psZ")
                    for t in range(TC):
                        for h in range(HPC):
                            nc.tensor.matmul(
                                Zq_ps[32 * h:32 * (h + 1), :],
                                lhsT=ones_sb[:, 0:32],
                                rhs=E_tiles[h][:, t, :],
                                start=(t == 0),
                                stop=(t == TC - 1),
                                skip_group_check=True,
                                tile_position=(0, 32 * h),
                            )

                    # Drain any leftover deferred matmuls: they fill the PE
                    # while DVE computes the reciprocal chain below.
                    steal(len(stolen))

                    rz_f32 = rzp.tile([P, NB], f32, tag="rz")
                    nc.vector.reciprocal_approx_fast(rz_f32, Zq_ps)
                    rz_bf = rzp.tile([P, NB], bf16, tag="rzc")
                    nc.vector.tensor_copy(rz_bf, rz_f32)

                    ATj = atp.tile([P, HPC, NB], bf16, tag="at")
                    for h in range(HPC):
                        # Broadcast strip h's reciprocal to all 128
                        # partitions with a K=32 ones-matmul (sums the 32
                        # identical rows), then fold the 1/32 and the
                        # normalize multiply into one fused DVE op reading
                        # the broadcast straight from PSUM.
                        rz_ps = psZ.tile([P, NB], f32, tag="psZ")
                        nc.tensor.matmul(
                            rz_ps,
                            lhsT=ones_sb[32 * h:32 * (h + 1), :],
                            rhs=rz_bf[32 * h:32 * (h + 1), :],
                            start=True,
                            stop=True,
                            tile_position=(32 * h, 0),
                        )
                        nc.vector.scalar_tensor_tensor(
                            ATj[:, h, :], OT_sb[:, h, :], 1.0 / 32.0, rz_ps,
                            Mult, Mult,
                        )
                    AT_prev = ATj
                    if j < SB - 1:
                        QT8j = QT8next

                # Final block's out-projection has nothing left to hide
                # behind; emit it directly.
                queue_outproj(AT_prev, SB - 1)
                steal(len(stolen))

    nc.compile()
    return nc


def _get_program():
    global _PROGRAM
    if _PROGRAM is None:
        _PROGRAM = build_program()
    return _PROGRAM


def make_in_maps(hidden_q, hidden_kv, attention_mask, position_bias, wq, wk, wv, wo):
    """Host-side shard + transpose + cast for all 8 cores."""
    f32 = np.float32

    def dxp(x):  # [n, (dc p)] -> [p, dc, n]  (transpose with d on partitions)
        n = x.shape[0]
        return np.ascontiguousarray(x.reshape(n, DC, P).transpose(2, 1, 0))

    def blocked(t):  # [p, dc, n] -> [SB, p, dc, NB]  (contiguous DMA slices)
        return np.ascontiguousarray(
            t.reshape(P, DC, SB, NB).transpose(2, 0, 1, 3)
        )

    hq8_b = [blocked(dxp(np.asarray(hidden_q[b], f32))).astype(FP8) for b in range(B)]
    hkv_t = [blocked(dxp(np.asarray(hidden_kv[b], f32))) for b in range(B)]
    hk8_b = [t.astype(FP8) for t in hkv_t]
    hkv_b = [t.astype(BF16) for t in hkv_t]

    mask = np.asarray(attention_mask)
    mask_all_ones = bool(mask.all())

    w_by_hg = []
    for hg in range(HPC):
        rows = slice(hg * OC, (hg + 1) * OC)
        wq8 = (dxp(np.asarray(wq[rows], f32)) * W8SCALE).astype(FP8)
        wk8 = (dxp(np.asarray(wk[rows], f32)) * W8SCALE).astype(FP8)
        wvT = dxp(np.asarray(wv[rows], f32)).astype(BF16)
        woT = np.ascontiguousarray(
            np.asarray(wo[:, rows], f32).reshape(DM, HPC, P).transpose(2, 1, 0)
        ).astype(BF16)
        w_by_hg.append((wq8, wk8, wvT, woT))

    in_maps = []
    for core in range(NCORES):
        b, hg = divmod(core, HPC)
        pb_sel = np.asarray(position_bias[hg * HPC:(hg + 1) * HPC], f32)
        pbT = pb_sel.reshape(HPC, LQ, TC, P).transpose(0, 3, 2, 1)  # [h,p,tc,s]
        pbe = np.exp(pbT, dtype=f32)
        if not mask_all_ones:
            # mask folded multiplicatively into exp(pb): zeroed keys drop out
            # of both the numerator and the softmax denominator, matching
            # where(mask, score, -inf) + where(mask, probs, 0).
            mT = mask[b].T.reshape(TC, P, LQ).transpose(1, 0, 2)
            pbe = pbe * mT[None].astype(f32)
        # block-major on s: [h, p, tc, s] -> [h, SB, p, tc, NB]
        pbe = np.ascontiguousarray(
            pbe.reshape(HPC, P, TC, SB, NB).transpose(0, 3, 1, 2, 4)
        )
        wq8, wk8, wvT, woT = w_by_hg[hg]
        in_maps.append(
            {
                "hq8": hq8_b[b],
                "hk8": hk8_b[b],
                "hkv": hkv_b[b],
                "wq8": wq8,
                "wk8": wk8,
                "wvT": wvT,
                "woT": woT,
                "pbe": pbe.astype(BF16),
            }
        )
    return in_maps


def gather_output(results):
    """Sum the 4 row-parallel partials per batch; un-permute to [B, LQ, DM]."""
    out = np.zeros((B, LQ, DM), np.float32)
    for core in range(NCORES):
        b = core // HPC
        part = results[core]["out"]  # [P, LQ//P, DM]
        out[b] += part.transpose(1, 0, 2).reshape(LQ, DM)
    return out


def kernel(hidden_q, hidden_kv, attention_mask, position_bias, wq, wk, wv, wo):
    global _LAST_RESULTS
    nc = _get_program()
    in_maps = make_in_maps(
        hidden_q, hidden_kv, attention_mask, position_bias, wq, wk, wv, wo
    )
    trace = os.environ.get("KERNEL_TRACE", "0") == "1"
    res = run_bass_kernel_spmd(
        nc,
        in_maps,
        core_ids=list(range(NCORES)),
        trace=trace,
        trace_cores=[0] if trace else None,
    )
    _LAST_RESULTS = res
    return gather_output(res.results)


# revision 14
# speedup vs baseline: 1.0538x; 1.0019x over previous
"""CPMAnt attention kernel for 8 TRN2 NeuronCores.

Sharding: 8 cores = 2 batches x 4 head-groups (4 heads each).
Each core computes its batch's QKV projections for its 4 heads, attention
with position bias, and a row-parallel partial of the output projection.
Host sums the 4 partials per batch (Megatron row-parallel reduce done on
host at gather time; no collectives needed).

Matmuls run in bf16 with f32 PSUM accumulation, except the Q/K projections
which run fp8-e4m3 DoubleRow (2 contraction chunks per matmul): the CPMAnt
scores (std ~4e-4 after scaling) are tiny against the position bias
(std ~1), so fp8 noise on Q/K is invisible in the output. Weights are
pre-scaled by 64 on the host to sit in fp8's normal range. KT/QT are also
STORED in fp8 (scores again), which halves their SBUF footprint and lets
the score matmuls run fp8xfp8 (same PE rate as bf16). V/attention/output-
projection stay bf16 (their error hits the output linearly).

Softmax denominator via column-tiled strip matmuls: all 4 heads of an
s-block keep their E = exp(S)*exp(pb) tiles alive; Z_h = ones^T E_h runs
as four concurrent M=32 column strips of the PE array (tile_position via
sliced PSUM base partitions), so the 4 heads' denominators cost ~one
matmul stream instead of four. Each head's reciprocal row is then
broadcast to 128 partitions with a tiny K=32 ones-matmul, and the
normalize multiplies OT (copied to SBUF bf16 per head) by it.

Transposed-operand formulation (no on-device transposes):
  KT8[o,t]  = wk8.T @ hk8      (fp8 DoubleRow, stored fp8)
  V  [t,o]  = hkvT.T @ wvT     (bf16)
  QT8[o,s]  = wq8.T @ hq8      (fp8 DoubleRow, stored fp8)
  ST [t,s]  = KT8_h.T @ QT8_h  (fp8 operands, bf16-rate)
  ET        = exp(ST*ES) * exp(pbT)    (ACT exp over chunk pairs, DVE mult)
  OT[o,s]  += V_h.T @ ET       -> OT_sb bf16
  Zq[32h,s] = ones32.T @ ET    (4 concurrent column strips, one PSUM bank)
  rz        = recip(Zq); rz128_h = ones.T @ rz_h (K=32 broadcast matmul)
  AT        = OT_sb * rz128
  out[s,m] += AT_h.T @ woT

DMA queue split: all input loads go through the Sync HWDGE ring (pure
prefetch FIFO), all output stores through GpSimd SWDGE, so stores waiting
on compute never head-of-line-block the next block's prefetches.
"""

import math
import os

import numpy as np
import ml_dtypes

import concourse.bass as bass
import concourse.bacc as bacc
import concourse.tile as tile
from concourse import mybir
from concourse.bass_utils import run_bass_kernel_spmd

BF16 = ml_dtypes.bfloat16
FP8 = mybir.dt.np(mybir.dt.float8e4)

# Problem shapes (hardcoded per contest contract).
B, LQ, LK = 2, 2048, 2048
DM, H, DH = 2048, 16, 128
P = 128            # partitions
NCORES = 8
HPC = 4            # heads per core
OC = HPC * DH      # 512 output-proj contraction per core
DC = DM // P       # 16 d-chunks
TC = LK // P       # 16 t-chunks
SB = 4             # s-blocks per 2048
NB = LQ // SB      # 512
NPAIR = TC // 2    # 8 score-chunk pairs per block

W8SCALE = 64.0     # host pre-scale for fp8 weights
KV_SCALE = 1.0 / math.sqrt(DM)
OUT_SCALE = 1.0 / math.sqrt(H * DH)
# KT8 = K_A * k_true, QT8 = Q_A * q_true / sqrt(DH); exp undoes K_A*Q_A.
K_A = 64.0
Q_A = 512.0
K8_COPY = K_A / (W8SCALE * math.sqrt(DM))
Q8_COPY = Q_A / (W8SCALE * math.sqrt(DM) * math.sqrt(DH))
EXP_SCALE = 1.0 / (K_A * Q_A)

_PROGRAM = None          # cached compiled Bass program
_LAST_RESULTS = None     # BassKernelResults from the most recent run


def build_program():
    f32 = mybir.dt.float32
    bf16 = mybir.dt.bfloat16
    f8 = mybir.dt.float8e4
    DR = mybir.MatmulPerfMode.DoubleRow
    nc = bacc.Bacc()

    # Streamed tensors are stored block-major so every DMA slice is fully
    # contiguous (8-16KB per-partition lines -> full HBM rate).
    hq8 = nc.dram_tensor("hq8", [SB, P, DC, NB], f8, kind="ExternalInput")
    hk8 = nc.dram_tensor("hk8", [SB, P, DC, NB], f8, kind="ExternalInput")
    hkv = nc.dram_tensor("hkv", [SB, P, DC, NB], bf16, kind="ExternalInput")
    wq8 = nc.dram_tensor("wq8", [P, DC, OC], f8, kind="ExternalInput")
    wk8 = nc.dram_tensor("wk8", [P, DC, OC], f8, kind="ExternalInput")
    wvT = nc.dram_tensor("wvT", [P, DC, OC], bf16, kind="ExternalInput")
    woT = nc.dram_tensor("woT", [P, HPC, DM], bf16, kind="ExternalInput")
    pbe = nc.dram_tensor("pbe", [HPC, SB, P, TC, NB], bf16, kind="ExternalInput")
    out = nc.dram_tensor("out", [P, LQ // P, DM], f32, kind="ExternalOutput")

    Copy = mybir.ActivationFunctionType.Copy
    Exp = mybir.ActivationFunctionType.Exp
    Mult = mybir.AluOpType.mult

    with tile.TileContext(nc) as tc:
        with (
            tc.tile_pool(name="persist", bufs=1) as persist,
            tc.tile_pool(name="kv", bufs=1) as kvp,
            tc.tile_pool(name="hq_s", bufs=2) as hqs,
        ):
            KT8 = kvp.tile([P, HPC, LK], f8)
            V = kvp.tile([P, TC, OC], bf16)

            def emit_hq_dma(j):
                # on the ACT HWDGE ring so pb loads on the Sync ring can
                # never head-of-line-block the next QT projection
                hq_sl = hqs.tile([P, DC, NB], f8, tag="hq", name="hq_sl")
                nc.scalar.dma_start(out=hq_sl, in_=hq8[j])
                return hq_sl

            # ---- KT / V projections (hidden_kv) ----
            with (
                tc.tile_pool(name="wkv", bufs=1) as wkvp,
                tc.tile_pool(name="h8s", bufs=3) as h8s,
                tc.tile_pool(name="hstream", bufs=2) as hs,
                tc.tile_pool(name="psA", bufs=6, space="PSUM") as psA,
            ):
                # Warmup matmuls: fill the cold-start DMA wait with junk PE
                # work so HAM unthrottles before the real stream begins.
                warm = persist.tile([P, P], bf16, name="warm")
                nc.vector.memset(warm, 0.0)
                wps = psA.tile([P, P], f32, tag="psA")
                NWARM = 168
                for i in range(NWARM):
                    nc.tensor.matmul(
                        wps, lhsT=warm, rhs=warm,
                        start=(i == 0), stop=(i == NWARM - 1),
                    )

                # K projections first: only 2MB of fp8 (wk8 + first slice) is
                # startup-critical; V's bf16 loads trail behind on the ring.
                wk_sb = wkvp.tile([P, DC, OC], f8)
                nc.sync.dma_start(out=wk_sb, in_=wk8[:])
                k_sl0 = h8s.tile([P, DC, NB], f8, tag="h8")
                nc.sync.dma_start(out=k_sl0, in_=hk8[0])
                ones_sb = persist.tile([P, P], bf16)
                nc.vector.memset(ones_sb, 1.0)
                wq_sb = persist.tile([P, DC, OC], f8)
                woT_sb = persist.tile([P, HPC, DM], bf16)
                hq_tiles = []

                for j in range(SB):
                    if j == 0:
                        k_sl = k_sl0
                    else:
                        k_sl = h8s.tile([P, DC, NB], f8, tag="h8")
                        nc.sync.dma_start(out=k_sl, in_=hk8[j])
                    for h in range(HPC):
                        ps = psA.tile([P, NB], f32, tag="psA")
                        for d in range(0, DC, 2):
                            nc.tensor.matmul(
                                ps,
                                lhsT=wk_sb[:, d:d + 2, h * P:(h + 1) * P],
                                rhs=k_sl[:, d:d + 2, :],
                                start=(d == 0),
                                stop=(d == DC - 2),
                                perf_mode=DR,
                            )
                        nc.scalar.activation(
                            KT8[:, h, j * NB:(j + 1) * NB], ps, Copy, scale=K8_COPY
                        )
                    if j == 0:
                        # Non-startup-critical loads go on the ACT HWDGE
                        # ring, emitted behind j0's KT copies so they don't
                        # steal HBM bandwidth from the first hidden slices.
                        hq_tiles += [emit_hq_dma(0), emit_hq_dma(1)]
                        nc.scalar.dma_start(out=wq_sb, in_=wq8[:])
                        nc.scalar.dma_start(out=woT_sb, in_=woT[:])

                wv_sb = wkvp.tile([P, DC, OC], bf16)
                nc.sync.dma_start(out=wv_sb, in_=wvT[:])
                for j in range(SB):
                    h_sl = hs.tile([P, DC, NB], bf16, tag="h")
                    nc.sync.dma_start(out=h_sl, in_=hkv[j])
                    for t4 in range(4):
                        ps = psA.tile([P, NB], f32, tag="psA")
                        for d in range(DC):
                            nc.tensor.matmul(
                                ps,
                                lhsT=h_sl[:, d, t4 * P:(t4 + 1) * P],
                                rhs=wv_sb[:, d, :],
                                start=(d == 0),
                                stop=(d == DC - 1),
                            )
                        nc.scalar.activation(
                            V[:, j * 4 + t4, :], ps, Copy, scale=KV_SCALE
                        )

            # ---- fused main loop over s-blocks ----
            with (
                tc.tile_pool(name="qt", bufs=2) as qtp,
                tc.tile_pool(name="at", bufs=1) as atp,
                tc.tile_pool(name="ot", bufs=1) as otp,
                tc.tile_pool(name="pb", bufs=3) as pbp,
                tc.tile_pool(name="es", bufs=2) as esp,
                tc.tile_pool(name="E", bufs=4) as Ep,
                tc.tile_pool(name="rz", bufs=1) as rzp,
                tc.tile_pool(name="cst", bufs=2) as csp,
                tc.tile_pool(name="psS", bufs=2, space="PSUM") as psS,
                tc.tile_pool(name="psOP", bufs=1, space="PSUM") as psOP,
                tc.tile_pool(name="psO", bufs=1, space="PSUM") as psO,
                tc.tile_pool(name="psZ", bufs=1, space="PSUM") as psZ,
            ):
                # PE work-stealing queue: single-matmul thunks of dependency-
                # free deferred work (previous block's out-projection, next
                # block's QT projection) that are woven between the S/AV
                # matmuls so PE never idles while ACT works through the exps.
                stolen = []

                def steal(n):
                    for _ in range(min(n, len(stolen))):
                        stolen.pop(0)()

                def queue_qt_proj(hq_sl):
                    """Queue the next block's QT projection; returns the
                    (not-yet-written) fp8 QT tile."""
                    QT8n = qtp.tile([P, HPC, NB], f8, tag="qt", name="QT8n")
                    for hp in range(HPC // 2):
                        ps = psOP.tile([P, 2 * NB], f32, tag="op", name="psq")
                        for h2 in range(2):
                            h = 2 * hp + h2
                            for d in range(0, DC, 2):
                                def mm(h=h, d=d, ps=ps, h2=h2):
                                    nc.tensor.matmul(
                                        ps[:, h2 * NB:(h2 + 1) * NB],
                                        lhsT=wq_sb[:, d:d + 2, h * P:(h + 1) * P],
                                        rhs=hq_sl[:, d:d + 2, :],
                                        start=(d == 0),
                                        stop=(d == DC - 2),
                                        perf_mode=DR,
                                        skip_group_check=True,
                                    )
                                stolen.append(mm)
                        prev = stolen.pop()

                        def last_mm(prev=prev, hp=hp, ps=ps):
                            prev()
                            nc.vector.tensor_scalar_mul(
                                QT8n[:, 2 * hp:2 * hp + 2, :],
                                ps.rearrange("p (c n) -> p c n", c=2),
                                Q8_COPY,
                            )
                        stolen.append(last_mm)
                    return QT8n

                def queue_outproj(ATj, j):
                    """Queue block j's out-projection (row-parallel partial)."""
                    for sc4 in range(NB // P):
                        sc = j * (NB // P) + sc4
                        for mbp in range(DM // NB // 2):
                            ps = psOP.tile([P, 2 * NB], f32, tag="op", name="psop")
                            for mb2 in range(2):
                                mb = 2 * mbp + mb2
                                for oc in range(HPC):
                                    def mm(ps=ps, mb2=mb2, mb=mb, oc=oc,
                                           ATj=ATj, sc4=sc4):
                                        nc.tensor.matmul(
                                            ps[:, mb2 * NB:(mb2 + 1) * NB],
                                            lhsT=ATj[:, oc, sc4 * P:(sc4 + 1) * P],
                                            rhs=woT_sb[:, oc, mb * NB:(mb + 1) * NB],
                                            start=(oc == 0),
                                            stop=(oc == HPC - 1),
                                            skip_group_check=True,
                                        )
                                    stolen.append(mm)
                            def store(ps=ps, sc=sc, mbp=mbp):
                                cst = csp.tile([P, 2 * NB], f32, tag="cs")
                                nc.vector.tensor_scalar_mul(cst, ps, OUT_SCALE)
                                nc.gpsimd.dma_start(
                                    out=out[:, sc, mbp * 2 * NB:(mbp + 1) * 2 * NB],
                                    in_=cst,
                                )
                            prev = stolen.pop()
                            def last_mm(prev=prev, store=store):
                                prev()
                                store()
                            stolen.append(last_mm)

                # Rolling position-bias prefetch, 2 blocks deep.
                blocks = [(j, h) for j in range(SB) for h in range(HPC)]

                def emit_pb_dma(j, h):
                    pb_sl = pbp.tile([P, TC, NB], bf16, tag="pb", name="pb_sl")
                    nc.sync.dma_start(out=pb_sl, in_=pbe[h, j])
                    return pb_sl

                pb_tiles = {bl: emit_pb_dma(*bl) for bl in blocks[:2]}

                # j=0's QT projection runs immediately (nothing to overlap).
                QT8j = queue_qt_proj(hq_tiles.pop(0))
                steal(len(stolen))
                hq_next = hq_tiles.pop(0)
                AT_prev = None
                for j in range(SB):
                    # Deferred work woven into this block's S/AV stream. QT
                    # for j+1 is queued FIRST: the psOP pool rotation makes
                    # queue order execution order, and j+1's score matmuls
                    # are blocked until its QT completes — it must clear
                    # early in the stream, not at the boundary.
                    if j < SB - 1:
                        QT8next = queue_qt_proj(hq_next)
                        if j < SB - 2:
                            hq_next = emit_hq_dma(j + 2)
                        elif j == SB - 2:
                            hq_next = None
                    if AT_prev is not None:
                        queue_outproj(AT_prev, j - 1)

                    OT_sb = otp.tile([P, HPC, NB], bf16, tag="ot")
                    E_tiles = []
                    for h in range(HPC):
                        pb_sl = pb_tiles.pop((j, h))
                        ahead = blocks.index((j, h)) + 2
                        if ahead < len(blocks):
                            pb_tiles[blocks[ahead]] = emit_pb_dma(*blocks[ahead])
                        E_sl = Ep.tile([P, TC, NB], bf16, tag="E")
                        E_tiles.append(E_sl)
                        O_ps = psO.tile([P, NB], f32, tag="psO")

                        def av(t):
                            nc.tensor.matmul(
                                O_ps,
                                lhsT=V[:, t, h * DH:(h + 1) * DH],
                                rhs=E_sl[:, t, :],
                                start=(t == 0),
                                stop=(t == TC - 1),
                                skip_group_check=True,
                            )

                        for p in range(NPAIR):
                            S_ps = psS.tile([P, 2 * NB], f32, tag="big")
                            for q in range(2):
                                nc.tensor.matmul(
                                    S_ps[:, q * NB:(q + 1) * NB],
                                    lhsT=KT8[:, h, (2 * p + q) * P:(2 * p + q + 1) * P],
                                    rhs=QT8j[:, h, :],
                                    start=True,
                                    stop=True,
                                    skip_group_check=True,
                                )
                            steal(1 if p >= 4 else 2)
                            eS = esp.tile([P, 2 * NB], bf16, tag="es")
                            nc.scalar.activation(eS, S_ps, Exp, scale=EXP_SCALE)
                            nc.vector.tensor_tensor(
                                E_sl[:, 2 * p:2 * p + 2, :],
                                eS.rearrange("p (c n) -> p c n", c=2),
                                pb_sl[:, 2 * p:2 * p + 2, :],
                                Mult,
                            )
                            if p >= 2:
                                av(2 * p - 4)
                                av(2 * p - 3)
                        for t in range(TC - 4, TC):
                            av(t)
                            steal(1)
                        # Free the PSUM bank for the next head; the end-of-
                        # block normalize reads the bf16 SBUF copy instead.
                        nc.vector.tensor_copy(OT_sb[:, h, :], O_ps)

                    # Softmax denominators for all 4 heads as concurrent
                    # 32-wide column strips: one matmul stream's worth of PE
                    # time instead of four.
                    Zq_ps = psZ.tile([P, NB], f32, tag="psZ")
                    for t in range(TC):
                        for h in range(HPC):
                            nc.tensor.matmul(
                                Zq_ps[32 * h:32 * (h + 1), :],
                                lhsT=ones_sb[:, 0:32],
                                rhs=E_tiles[h][:, t, :],
                                start=(t == 0),
                                stop=(t == TC - 1),
                                skip_group_check=True,
                                tile_position=(0, 32 * h),
                            )

                    # Drain any leftover deferred matmuls: they fill the PE
                    # while DVE computes the reciprocal chain below.
                    steal(len(stolen))

                    rz_f32 = rzp.tile([P, NB], f32, tag="rz")
                    nc.vector.reciprocal_approx_fast(rz_f32, Zq_ps)
                    rz_bf = rzp.tile([P, NB], bf16, tag="rzc")
                    nc.vector.tensor_copy(rz_bf, rz_f32)

                    ATj = atp.tile([P, HPC, NB], bf16, tag="at")
                    for h in range(HPC):
                        # Broadcast strip h's reciprocal to all 128
                        # partitions with a K=32 ones-matmul (sums the 32
                        # identical rows), then fold the 1/32 and the
                        # normalize multiply into one fused DVE op reading
                        # the broadcast straight from PSUM.
                        rz_ps = psZ.tile([P, NB], f32, tag="psZ")
                        nc.tensor.matmul(
                            rz_ps,
                            lhsT=ones_sb[32 * h:32 * (h + 1), :],
                            rhs=rz_bf[32 * h:32 * (h + 1), :],
                            start=True,
                            stop=True,
                            tile_position=(32 * h, 0),
                        )
                        nc.vector.scalar_tensor_tensor(
                            ATj[:, h, :], OT_sb[:, h, :], 1.0 / 32.0, rz_ps,
                            Mult, Mult,
                        )
                    AT_prev = ATj
                    if j < SB - 1:
                        QT8j = QT8next

                # Final block's out-projection has nothing left to hide
                # behind; emit it directly, double-buffered through the psS
                # banks (free by now) so the DVE scale of tile i overlaps
                # the matmuls of tile i+1.
                for sc4 in range(NB // P):
                    sc = (SB - 1) * (NB // P) + sc4
                    for mbp in range(DM // NB // 2):
                        ps = psS.tile([P, 2 * NB], f32, tag="big")
                        for mb2 in range(2):
                            mb = 2 * mbp + mb2
                            for oc in range(HPC):
                                nc.tensor.matmul(
                                    ps[:, mb2 * NB:(mb2 + 1) * NB],
                                    lhsT=AT_prev[:, oc, sc4 * P:(sc4 + 1) * P],
                                    rhs=woT_sb[:, oc, mb * NB:(mb + 1) * NB],
                                    start=(oc == 0),
                                    stop=(oc == HPC - 1),
                                    skip_group_check=True,
                                )
                        cst = csp.tile([P, 2 * NB], f32, tag="cs")
                        nc.vector.tensor_scalar_mul(cst, ps, OUT_SCALE)
                        nc.gpsimd.dma_start(
                            out=out[:, sc, mbp * 2 * NB:(mbp + 1) * 2 * NB],
                            in_=cst,
                        )

    nc.compile()
    return nc


def _get_program():
    global _PROGRAM
    if _PROGRAM is None:
        _PROGRAM = build_program()
    return _PROGRAM


def make_in_maps(hidden_q, hidden_kv, attention_mask, position_bias, wq, wk, wv, wo):
    """Host-side shard + transpose + cast for all 8 cores."""
    f32 = np.float32

    def dxp(x):  # [n, (dc p)] -> [p, dc, n]  (transpose with d on partitions)
        n = x.shape[0]
        return np.ascontiguousarray(x.reshape(n, DC, P).transpose(2, 1, 0))

    def blocked(t):  # [p, dc, n] -> [SB, p, dc, NB]  (contiguous DMA slices)
        return np.ascontiguousarray(
            t.reshape(P, DC, SB, NB).transpose(2, 0, 1, 3)
        )

    hq8_b = [blocked(dxp(np.asarray(hidden_q[b], f32))).astype(FP8) for b in range(B)]
    hkv_t = [blocked(dxp(np.asarray(hidden_kv[b], f32))) for b in range(B)]
    hk8_b = [t.astype(FP8) for t in hkv_t]
    hkv_b = [t.astype(BF16) for t in hkv_t]

    mask = np.asarray(attention_mask)
    mask_all_ones = bool(mask.all())

    w_by_hg = []
    for hg in range(HPC):
        rows = slice(hg * OC, (hg + 1) * OC)
        wq8 = (dxp(np.asarray(wq[rows], f32)) * W8SCALE).astype(FP8)
        wk8 = (dxp(np.asarray(wk[rows], f32)) * W8SCALE).astype(FP8)
        wvT = dxp(np.asarray(wv[rows], f32)).astype(BF16)
        woT = np.ascontiguousarray(
            np.asarray(wo[:, rows], f32).reshape(DM, HPC, P).transpose(2, 1, 0)
        ).astype(BF16)
        w_by_hg.append((wq8, wk8, wvT, woT))

    in_maps = []
    for core in range(NCORES):
        b, hg = divmod(core, HPC)
        pb_sel = np.asarray(position_bias[hg * HPC:(hg + 1) * HPC], f32)
        pbT = pb_sel.reshape(HPC, LQ, TC, P).transpose(0, 3, 2, 1)  # [h,p,tc,s]
        pbe = np.exp(pbT, dtype=f32)
        if not mask_all_ones:
            # mask folded multiplicatively into exp(pb): zeroed keys drop out
            # of both the numerator and the softmax denominator, matching
            # where(mask, score, -inf) + where(mask, probs, 0).
            mT = mask[b].T.reshape(TC, P, LQ).transpose(1, 0, 2)
            pbe = pbe * mT[None].astype(f32)
        # block-major on s: [h, p, tc, s] -> [h, SB, p, tc, NB]
        pbe = np.ascontiguousarray(
            pbe.reshape(HPC, P, TC, SB, NB).transpose(0, 3, 1, 2, 4)
        )
        wq8, wk8, wvT, woT = w_by_hg[hg]
        in_maps.append(
            {
                "hq8": hq8_b[b],
                "hk8": hk8_b[b],
                "hkv": hkv_b[b],
                "wq8": wq8,
                "wk8": wk8,
                "wvT": wvT,
                "woT": woT,
                "pbe": pbe.astype(BF16),
            }
        )
    return in_maps


def gather_output(results):
    """Sum the 4 row-parallel partials per batch; un-permute to [B, LQ, DM]."""
    out = np.zeros((B, LQ, DM), np.float32)
    for core in range(NCORES):
        b = core // HPC
        part = results[core]["out"]  # [P, LQ//P, DM]
        out[b] += part.transpose(1, 0, 2).reshape(LQ, DM)
    return out


def kernel(hidden_q, hidden_kv, attention_mask, position_bias, wq, wk, wv, wo):
    global _LAST_RESULTS
    nc = _get_program()
    in_maps = make_in_maps(
        hidden_q, hidden_kv, attention_mask, position_bias, wq, wk, wv, wo
    )
    trace = os.environ.get("KERNEL_TRACE", "0") == "1"
    res = run_bass_kernel_spmd(
        nc,
        in_maps,
        core_ids=list(range(NCORES)),
        trace=trace,
        trace_cores=[0] if trace else None,
    )
    _LAST_RESULTS = res
    return gather_output(res.results)


# revision 18
# speedup vs baseline: 1.0782x; 1.0231x over previous
"""CPMAnt attention kernel for 8 TRN2 NeuronCores.

Sharding: 8 cores = 2 batches x 4 head-groups (4 heads each).
Each core computes its batch's QKV projections for its 4 heads, attention
with position bias, and a row-parallel partial of the output projection.
Host sums the 4 partials per batch (Megatron row-parallel reduce done on
host at gather time; no collectives needed).

Matmuls run in bf16 with f32 PSUM accumulation, except the Q/K projections
which run fp8-e4m3 DoubleRow (2 contraction chunks per matmul): the CPMAnt
scores (std ~4e-4 after scaling) are tiny against the position bias
(std ~1), so fp8 noise on Q/K is invisible in the output. Weights are
pre-scaled by 64 on the host to sit in fp8's normal range. KT/QT are also
STORED in fp8 (scores again), which halves their SBUF footprint and lets
the score matmuls run fp8xfp8 (same PE rate as bf16). V/attention/output-
projection stay bf16 (their error hits the output linearly).

Softmax denominator via column-tiled strip matmuls: all 4 heads of an
s-block keep their E = exp(S)*exp(pb) tiles alive; Z_h = ones^T E_h runs
as four concurrent M=32 column strips of the PE array (tile_position via
sliced PSUM base partitions), so the 4 heads' denominators cost ~one
matmul stream instead of four. Each head's reciprocal row is then
broadcast to 128 partitions with a tiny K=32 ones-matmul, and the
normalize multiplies OT (copied to SBUF bf16 per head) by it.

Transposed-operand formulation (no on-device transposes):
  KT8[o,t]  = wk8.T @ hk8      (fp8 DoubleRow, stored fp8)
  V  [t,o]  = hkvT.T @ wvT     (bf16)
  QT8[o,s]  = wq8.T @ hq8      (fp8 DoubleRow, stored fp8)
  ST [t,s]  = KT8_h.T @ QT8_h  (fp8 operands, bf16-rate)
  ET        = exp(ST*ES) * exp(pbT)    (ACT exp over chunk pairs, DVE mult)
  OT[o,s]  += V_h.T @ ET       -> OT_sb bf16
  Zq[32h,s] = ones32.T @ ET    (4 concurrent column strips, one PSUM bank)
  rz        = recip(Zq); rz128_h = ones.T @ rz_h (K=32 broadcast matmul)
  AT        = OT_sb * rz128
  out[s,m] += AT_h.T @ woT

DMA queue split: all input loads go through the Sync HWDGE ring (pure
prefetch FIFO), all output stores through GpSimd SWDGE, so stores waiting
on compute never head-of-line-block the next block's prefetches.
"""

import math
import os

import numpy as np
import ml_dtypes

import concourse.bass as bass
import concourse.bacc as bacc
import concourse.tile as tile
from concourse import mybir
from concourse.bass_utils import run_bass_kernel_spmd

BF16 = ml_dtypes.bfloat16
FP8 = mybir.dt.np(mybir.dt.float8e4)

# Problem shapes (hardcoded per contest contract).
B, LQ, LK = 2, 2048, 2048
DM, H, DH = 2048, 16, 128
P = 128            # partitions
NCORES = 8
HPC = 4            # heads per core
OC = HPC * DH      # 512 output-proj contraction per core
DC = DM // P       # 16 d-chunks
TC = LK // P       # 16 t-chunks
SB = 4             # s-blocks per 2048
NB = LQ // SB      # 512
NPAIR = TC // 2    # 8 score-chunk pairs per block

W8SCALE = 64.0     # host pre-scale for fp8 weights
KV_SCALE = 1.0 / math.sqrt(DM)
OUT_SCALE = 1.0 / math.sqrt(H * DH)
# KT8 = K_A * k_true, QT8 = Q_A * q_true / sqrt(DH); exp undoes K_A*Q_A.
K_A = 64.0
Q_A = 512.0
K8_COPY = K_A / (W8SCALE * math.sqrt(DM))
Q8_COPY = Q_A / (W8SCALE * math.sqrt(DM) * math.sqrt(DH))
EXP_SCALE = 1.0 / (K_A * Q_A)

_PROGRAM = None          # cached compiled Bass program
_LAST_RESULTS = None     # BassKernelResults from the most recent run


def build_program():
    f32 = mybir.dt.float32
    bf16 = mybir.dt.bfloat16
    f8 = mybir.dt.float8e4
    DR = mybir.MatmulPerfMode.DoubleRow
    nc = bacc.Bacc()

    # Streamed tensors are stored block-major so every DMA slice is fully
    # contiguous (8-16KB per-partition lines -> full HBM rate).
    hq8 = nc.dram_tensor("hq8", [SB, P, DC, NB], f8, kind="ExternalInput")
    hk8 = nc.dram_tensor("hk8", [SB, P, DC, NB], f8, kind="ExternalInput")
    hkv = nc.dram_tensor("hkv", [SB, P, DC, NB], bf16, kind="ExternalInput")
    wq8 = nc.dram_tensor("wq8", [P, DC, OC], f8, kind="ExternalInput")
    wk8 = nc.dram_tensor("wk8", [P, DC, OC], f8, kind="ExternalInput")
    wvT = nc.dram_tensor("wvT", [P, DC, OC], bf16, kind="ExternalInput")
    woT = nc.dram_tensor("woT", [P, HPC, DM], bf16, kind="ExternalInput")
    pbe = nc.dram_tensor("pbe", [HPC, SB, P, TC, NB], bf16, kind="ExternalInput")
    out = nc.dram_tensor("out", [P, LQ // P, DM], f32, kind="ExternalOutput")

    Copy = mybir.ActivationFunctionType.Copy
    Exp = mybir.ActivationFunctionType.Exp
    Mult = mybir.AluOpType.mult

    with tile.TileContext(nc) as tc:
        with (
            tc.tile_pool(name="persist", bufs=1) as persist,
            tc.tile_pool(name="kv", bufs=1) as kvp,
            tc.tile_pool(name="h8s", bufs=3) as h8s,
        ):
            KT8 = kvp.tile([P, HPC, LK], f8)
            V = kvp.tile([P, TC, OC], bf16)

            def emit_hq_dma(j):
                # Allocated from the same rotating pool as the hk slices:
                # the pool WAR semaphore gates this DMA until a Kproj block
                # has consumed its buffer, so these non-startup-critical
                # loads can't steal HBM bandwidth from the critical stream.
                # (ACT HWDGE ring, so pb loads on the Sync ring can never
                # head-of-line-block the next QT projection either.)
                hq_sl = h8s.tile([P, DC, NB], f8, tag="h8", name="hq_sl")
                nc.scalar.dma_start(out=hq_sl, in_=hq8[j])
                return hq_sl

            # ---- KT / V projections (hidden_kv) ----
            with (
                tc.tile_pool(name="wkv", bufs=1) as wkvp,
                tc.tile_pool(name="hstream", bufs=2) as hs,
                tc.tile_pool(name="psA", bufs=6, space="PSUM") as psA,
            ):
                # Warmup matmuls: fill the cold-start DMA wait with junk PE
                # work so HAM unthrottles before the real stream begins.
                warm = persist.tile([P, P], bf16, name="warm")
                nc.vector.memset(warm, 0.0)
                wps = psA.tile([P, P], f32, tag="psA")
                NWARM = 168
                for i in range(NWARM):
                    nc.tensor.matmul(
                        wps, lhsT=warm, rhs=warm,
                        start=(i == 0), stop=(i == NWARM - 1),
                    )

                # K projections first: only 2MB of fp8 (wk8 + first slice) is
                # startup-critical; V's bf16 loads trail behind on the ring.
                wk_sb = wkvp.tile([P, DC, OC], f8)
                nc.sync.dma_start(out=wk_sb, in_=wk8[:])
                k_sl0 = h8s.tile([P, DC, NB], f8, tag="h8")
                nc.sync.dma_start(out=k_sl0, in_=hk8[0])
                ones_sb = persist.tile([P, P], bf16)
                nc.vector.memset(ones_sb, 1.0)
                woT_sb = persist.tile([P, HPC, DM], bf16)
                # woT is the only ungated early load on the ACT ring; it
                # streams while the Sync ring feeds the K projections.
                nc.scalar.dma_start(out=woT_sb, in_=woT[:])
                hq_tiles = []

                for j in range(SB):
                    if j == 0:
                        k_sl = k_sl0
                    else:
                        k_sl = h8s.tile([P, DC, NB], f8, tag="h8")
                        nc.sync.dma_start(out=k_sl, in_=hk8[j])
                    for h in range(HPC):
                        ps = psA.tile([P, NB], f32, tag="psA")
                        for d in range(0, DC, 2):
                            nc.tensor.matmul(
                                ps,
                                lhsT=wk_sb[:, d:d + 2, h * P:(h + 1) * P],
                                rhs=k_sl[:, d:d + 2, :],
                                start=(d == 0),
                                stop=(d == DC - 2),
                                perf_mode=DR,
                            )
                        nc.scalar.activation(
                            KT8[:, h, j * NB:(j + 1) * NB], ps, Copy, scale=K8_COPY
                        )
                # hq / wq ride the h8s pool rotation: each DMA is WAR-gated
                # on a consumed hk buffer (Kproj j1/j2/j3 done), keeping the
                # early HBM bandwidth for the startup-critical stream.
                hq_tiles += [emit_hq_dma(0), emit_hq_dma(1)]
                wq_sb = h8s.tile([P, DC, OC], f8, tag="h8", name="wq_sb")
                nc.scalar.dma_start(out=wq_sb, in_=wq8[:])

                wv_sb = wkvp.tile([P, DC, OC], bf16)
                nc.sync.dma_start(out=wv_sb, in_=wvT[:])
                for j in range(SB):
                    h_sl = hs.tile([P, DC, NB], bf16, tag="h")
                    nc.sync.dma_start(out=h_sl, in_=hkv[j])
                    for t4 in range(4):
                        ps = psA.tile([P, NB], f32, tag="psA")
                        for d in range(DC):
                            nc.tensor.matmul(
                                ps,
                                lhsT=h_sl[:, d, t4 * P:(t4 + 1) * P],
                                rhs=wv_sb[:, d, :],
                                start=(d == 0),
                                stop=(d == DC - 1),
                            )
                        nc.scalar.activation(
                            V[:, j * 4 + t4, :], ps, Copy, scale=KV_SCALE
                        )

            # ---- fused main loop over s-blocks ----
            with (
                tc.tile_pool(name="qt", bufs=2) as qtp,
                tc.tile_pool(name="at", bufs=1) as atp,
                tc.tile_pool(name="ot", bufs=1) as otp,
                tc.tile_pool(name="pb", bufs=3) as pbp,
                tc.tile_pool(name="es", bufs=2) as esp,
                tc.tile_pool(name="E", bufs=4) as Ep,
                tc.tile_pool(name="rz", bufs=1) as rzp,
                tc.tile_pool(name="cst", bufs=2) as csp,
                tc.tile_pool(name="psS", bufs=2, space="PSUM") as psS,
                tc.tile_pool(name="psOP", bufs=1, space="PSUM") as psOP,
                tc.tile_pool(name="psO", bufs=1, space="PSUM") as psO,
                tc.tile_pool(name="psZ", bufs=1, space="PSUM") as psZ,
            ):
                # PE work-stealing queue: single-matmul thunks of dependency-
                # free deferred work (previous block's out-projection, next
                # block's QT projection) that are woven between the S/AV
                # matmuls so PE never idles while ACT works through the exps.
                stolen = []

                def steal(n):
                    for _ in range(min(n, len(stolen))):
                        stolen.pop(0)()

                def queue_qt_proj(hq_sl):
                    """Queue the next block's QT projection; returns the
                    (not-yet-written) fp8 QT tile."""
                    QT8n = qtp.tile([P, HPC, NB], f8, tag="qt", name="QT8n")
                    for hp in range(HPC // 2):
                        ps = psOP.tile([P, 2 * NB], f32, tag="op", name="psq")
                        for h2 in range(2):
                            h = 2 * hp + h2
                            for d in range(0, DC, 2):
                                def mm(h=h, d=d, ps=ps, h2=h2):
                                    nc.tensor.matmul(
                                        ps[:, h2 * NB:(h2 + 1) * NB],
                                        lhsT=wq_sb[:, d:d + 2, h * P:(h + 1) * P],
                                        rhs=hq_sl[:, d:d + 2, :],
                                        start=(d == 0),
                                        stop=(d == DC - 2),
                                        perf_mode=DR,
                                        skip_group_check=True,
                                    )
                                stolen.append(mm)
                        prev = stolen.pop()

                        def last_mm(prev=prev, hp=hp, ps=ps):
                            prev()
                            nc.vector.tensor_scalar_mul(
                                QT8n[:, 2 * hp:2 * hp + 2, :],
                                ps.rearrange("p (c n) -> p c n", c=2),
                                Q8_COPY,
                            )
                        stolen.append(last_mm)
                    return QT8n

                def queue_outproj(ATj, j):
                    """Queue block j's out-projection (row-parallel partial)."""
                    for sc4 in range(NB // P):
                        sc = j * (NB // P) + sc4
                        for mbp in range(DM // NB // 2):
                            ps = psOP.tile([P, 2 * NB], f32, tag="op", name="psop")
                            for mb2 in range(2):
                                mb = 2 * mbp + mb2
                                for oc in range(HPC):
                                    def mm(ps=ps, mb2=mb2, mb=mb, oc=oc,
                                           ATj=ATj, sc4=sc4):
                                        nc.tensor.matmul(
                                            ps[:, mb2 * NB:(mb2 + 1) * NB],
                                            lhsT=ATj[:, oc, sc4 * P:(sc4 + 1) * P],
                                            rhs=woT_sb[:, oc, mb * NB:(mb + 1) * NB],
                                            start=(oc == 0),
                                            stop=(oc == HPC - 1),
                                            skip_group_check=True,
                                        )
                                    stolen.append(mm)
                            def store(ps=ps, sc=sc, mbp=mbp):
                                cst = csp.tile([P, 2 * NB], f32, tag="cs")
                                nc.vector.tensor_scalar_mul(cst, ps, OUT_SCALE)
                                nc.gpsimd.dma_start(
                                    out=out[:, sc, mbp * 2 * NB:(mbp + 1) * 2 * NB],
                                    in_=cst,
                                )
                            prev = stolen.pop()
                            def last_mm(prev=prev, store=store):
                                prev()
                                store()
                            stolen.append(last_mm)

                # Rolling position-bias prefetch, 2 blocks deep.
                blocks = [(j, h) for j in range(SB) for h in range(HPC)]

                def emit_pb_dma(j, h):
                    pb_sl = pbp.tile([P, TC, NB], bf16, tag="pb", name="pb_sl")
                    nc.sync.dma_start(out=pb_sl, in_=pbe[h, j])
                    return pb_sl

                pb_tiles = {bl: emit_pb_dma(*bl) for bl in blocks[:2]}

                # j=0's QT projection runs immediately (nothing to overlap).
                QT8j = queue_qt_proj(hq_tiles.pop(0))
                steal(len(stolen))
                hq_next = hq_tiles.pop(0)
                AT_prev = None
                for j in range(SB):
                    # Deferred work woven into this block's S/AV stream. QT
                    # for j+1 is queued FIRST: the psOP pool rotation makes
                    # queue order execution order, and j+1's score matmuls
                    # are blocked until its QT completes — it must clear
                    # early in the stream, not at the boundary.
                    if j < SB - 1:
                        QT8next = queue_qt_proj(hq_next)
                        if j < SB - 2:
                            hq_next = emit_hq_dma(j + 2)
                        elif j == SB - 2:
                            hq_next = None
                    if AT_prev is not None:
                        queue_outproj(AT_prev, j - 1)

                    OT_sb = otp.tile([P, HPC, NB], bf16, tag="ot")
                    E_tiles = []
                    for h in range(HPC):
                        pb_sl = pb_tiles.pop((j, h))
                        ahead = blocks.index((j, h)) + 2
                        if ahead < len(blocks):
                            pb_tiles[blocks[ahead]] = emit_pb_dma(*blocks[ahead])
                        E_sl = Ep.tile([P, TC, NB], bf16, tag="E")
                        E_tiles.append(E_sl)
                        O_ps = psO.tile([P, NB], f32, tag="psO")

                        def av(t):
                            nc.tensor.matmul(
                                O_ps,
                                lhsT=V[:, t, h * DH:(h + 1) * DH],
                                rhs=E_sl[:, t, :],
                                start=(t == 0),
                                stop=(t == TC - 1),
                                skip_group_check=True,
                            )

                        for p in range(NPAIR):
                            S_ps = psS.tile([P, 2 * NB], f32, tag="big")
                            for q in range(2):
                                nc.tensor.matmul(
                                    S_ps[:, q * NB:(q + 1) * NB],
                                    lhsT=KT8[:, h, (2 * p + q) * P:(2 * p + q + 1) * P],
                                    rhs=QT8j[:, h, :],
                                    start=True,
                                    stop=True,
                                    skip_group_check=True,
                                )
                            steal(1 if p >= 4 else 2)
                            eS = esp.tile([P, 2 * NB], bf16, tag="es")
                            nc.scalar.activation(eS, S_ps, Exp, scale=EXP_SCALE)
                            nc.vector.tensor_tensor(
                                E_sl[:, 2 * p:2 * p + 2, :],
                                eS.rearrange("p (c n) -> p c n", c=2),
                                pb_sl[:, 2 * p:2 * p + 2, :],
                                Mult,
                            )
                            if p >= 2:
                                av(2 * p - 4)
                                av(2 * p - 3)
                        for t in range(TC - 4, TC):
                            av(t)
                            steal(1)
                        # Free the PSUM bank for the next head; the end-of-
                        # block normalize reads the bf16 SBUF copy instead.
                        nc.vector.tensor_copy(OT_sb[:, h, :], O_ps)

                    # Softmax denominators for all 4 heads as concurrent
                    # 32-wide column strips: one matmul stream's worth of PE
                    # time instead of four.
                    Zq_ps = psZ.tile([P, NB], f32, tag="psZ")
                    for t in range(TC):
                        for h in range(HPC):
                            nc.tensor.matmul(
                                Zq_ps[32 * h:32 * (h + 1), :],
                                lhsT=ones_sb[:, 0:32],
                                rhs=E_tiles[h][:, t, :],
                                start=(t == 0),
                                stop=(t == TC - 1),
                                skip_group_check=True,
                                tile_position=(0, 32 * h),
                            )

                    # Drain any leftover deferred matmuls: they fill the PE
                    # while DVE computes the reciprocal chain below.
                    steal(len(stolen))

                    rz_f32 = rzp.tile([P, NB], f32, tag="rz")
                    nc.vector.reciprocal_approx_fast(rz_f32, Zq_ps)
                    rz_bf = rzp.tile([P, NB], bf16, tag="rzc")
                    nc.vector.tensor_copy(rz_bf, rz_f32)

                    ATj = atp.tile([P, HPC, NB], bf16, tag="at")
                    for h in range(HPC):
                        # Broadcast strip h's reciprocal to all 128
                        # partitions with a K=32 ones-matmul (sums the 32
                        # identical rows), then fold the 1/32 and the
                        # normalize multiply into one fused DVE op reading
                        # the broadcast straight from PSUM.
                        rz_ps = psZ.tile([P, NB], f32, tag="psZ")
                        nc.tensor.matmul(
                            rz_ps,
                            lhsT=ones_sb[32 * h:32 * (h + 1), :],
                            rhs=rz_bf[32 * h:32 * (h + 1), :],
                            start=True,
                            stop=True,
                            tile_position=(32 * h, 0),
                        )
                        nc.vector.scalar_tensor_tensor(
                            ATj[:, h, :], OT_sb[:, h, :], 1.0 / 32.0, rz_ps,
                            Mult, Mult,
                        )
                    AT_prev = ATj
                    if j < SB - 1:
                        QT8j = QT8next

                # Final block's out-projection has nothing left to hide
                # behind; emit it directly, double-buffered through the psS
                # banks (free by now) so the DVE scale of tile i overlaps
                # the matmuls of tile i+1.
                for sc4 in range(NB // P):
                    sc = (SB - 1) * (NB // P) + sc4
                    for mbp in range(DM // NB // 2):
                        ps = psS.tile([P, 2 * NB], f32, tag="big")
                        for mb2 in range(2):
                            mb = 2 * mbp + mb2
                            for oc in range(HPC):
                                nc.tensor.matmul(
                                    ps[:, mb2 * NB:(mb2 + 1) * NB],
                                    lhsT=AT_prev[:, oc, sc4 * P:(sc4 + 1) * P],
                                    rhs=woT_sb[:, oc, mb * NB:(mb + 1) * NB],
                                    start=(oc == 0),
                                    stop=(oc == HPC - 1),
                                    skip_group_check=True,
                                )
                        cst = csp.tile([P, 2 * NB], f32, tag="cs")
                        nc.vector.tensor_scalar_mul(cst, ps, OUT_SCALE)
                        nc.gpsimd.dma_start(
                            out=out[:, sc, mbp * 2 * NB:(mbp + 1) * 2 * NB],
                            in_=cst,
                        )

    nc.compile()
    return nc


def _get_program():
    global _PROGRAM
    if _PROGRAM is None:
        _PROGRAM = build_program()
    return _PROGRAM


def make_in_maps(hidden_q, hidden_kv, attention_mask, position_bias, wq, wk, wv, wo):
    """Host-side shard + transpose + cast for all 8 cores."""
    f32 = np.float32

    def dxp(x):  # [n, (dc p)] -> [p, dc, n]  (transpose with d on partitions)
        n = x.shape[0]
        return np.ascontiguousarray(x.reshape(n, DC, P).transpose(2, 1, 0))

    def blocked(t):  # [p, dc, n] -> [SB, p, dc, NB]  (contiguous DMA slices)
        return np.ascontiguousarray(
            t.reshape(P, DC, SB, NB).transpose(2, 0, 1, 3)
        )

    hq8_b = [blocked(dxp(np.asarray(hidden_q[b], f32))).astype(FP8) for b in range(B)]
    hkv_t = [blocked(dxp(np.asarray(hidden_kv[b], f32))) for b in range(B)]
    hk8_b = [t.astype(FP8) for t in hkv_t]
    hkv_b = [t.astype(BF16) for t in hkv_t]

    mask = np.asarray(attention_mask)
    mask_all_ones = bool(mask.all())

    w_by_hg = []
    for hg in range(HPC):
        rows = slice(hg * OC, (hg + 1) * OC)
        wq8 = (dxp(np.asarray(wq[rows], f32)) * W8SCALE).astype(FP8)
        wk8 = (dxp(np.asarray(wk[rows], f32)) * W8SCALE).astype(FP8)
        wvT = dxp(np.asarray(wv[rows], f32)).astype(BF16)
        woT = np.ascontiguousarray(
            np.asarray(wo[:, rows], f32).reshape(DM, HPC, P).transpose(2, 1, 0)
        ).astype(BF16)
        w_by_hg.append((wq8, wk8, wvT, woT))

    in_maps = []
    for core in range(NCORES):
        b, hg = divmod(core, HPC)
        pb_sel = np.asarray(position_bias[hg * HPC:(hg + 1) * HPC], f32)
        pbT = pb_sel.reshape(HPC, LQ, TC, P).transpose(0, 3, 2, 1)  # [h,p,tc,s]
        pbe = np.exp(pbT, dtype=f32)
        if not mask_all_ones:
            # mask folded multiplicatively into exp(pb): zeroed keys drop out
            # of both the numerator and the softmax denominator, matching
            # where(mask, score, -inf) + where(mask, probs, 0).
            mT = mask[b].T.reshape(TC, P, LQ).transpose(1, 0, 2)
            pbe = pbe * mT[None].astype(f32)
        # block-major on s: [h, p, tc, s] -> [h, SB, p, tc, NB]
        pbe = np.ascontiguousarray(
            pbe.reshape(HPC, P, TC, SB, NB).transpose(0, 3, 1, 2, 4)
        )
        wq8, wk8, wvT, woT = w_by_hg[hg]
        in_maps.append(
            {
                "hq8": hq8_b[b],
                "hk8": hk8_b[b],
                "hkv": hkv_b[b],
                "wq8": wq8,
                "wk8": wk8,
                "wvT": wvT,
                "woT": woT,
                "pbe": pbe.astype(BF16),
            }
        )
    return in_maps


def gather_output(results):
    """Sum the 4 row-parallel partials per batch; un-permute to [B, LQ, DM]."""
    out = np.zeros((B, LQ, DM), np.float32)
    for core in range(NCORES):
        b = core // HPC
        part = results[core]["out"]  # [P, LQ//P, DM]
        out[b] += part.transpose(1, 0, 2).reshape(LQ, DM)
    return out


def kernel(hidden_q, hidden_kv, attention_mask, position_bias, wq, wk, wv, wo):
    global _LAST_RESULTS
    nc = _get_program()
    in_maps = make_in_maps(
        hidden_q, hidden_kv, attention_mask, position_bias, wq, wk, wv, wo
    )
    trace = os.environ.get("KERNEL_TRACE", "0") == "1"
    res = run_bass_kernel_spmd(
        nc,
        in_maps,
        core_ids=list(range(NCORES)),
        trace=trace,
        trace_cores=[0] if trace else None,
    )
    _LAST_RESULTS = res
    return gather_output(res.results)


# revision 20
# speedup vs baseline: 1.0915x; 1.0123x over previous
"""CPMAnt attention kernel for 8 TRN2 NeuronCores.

Sharding: 8 cores = 2 batches x 4 head-groups (4 heads each).
Each core computes its batch's QKV projections for its 4 heads, attention
with position bias, and a row-parallel partial of the output projection.
Host sums the 4 partials per batch (Megatron row-parallel reduce done on
host at gather time; no collectives needed).

Matmuls run in bf16 with f32 PSUM accumulation, except the Q/K projections
which run fp8-e4m3 DoubleRow (2 contraction chunks per matmul): the CPMAnt
scores (std ~4e-4 after scaling) are tiny against the position bias
(std ~1), so fp8 noise on Q/K is invisible in the output. Weights are
pre-scaled by 64 on the host to sit in fp8's normal range. KT/QT are also
STORED in fp8 (scores again), which halves their SBUF footprint and lets
the score matmuls run fp8xfp8 (same PE rate as bf16). V/attention/output-
projection stay bf16 (their error hits the output linearly).

Softmax denominator via column-tiled strip matmuls: all 4 heads of an
s-block keep their E = exp(S)*exp(pb) tiles alive; Z_h = ones^T E_h runs
as four concurrent M=32 column strips of the PE array (tile_position via
sliced PSUM base partitions), so the 4 heads' denominators cost ~one
matmul stream instead of four. Each head's reciprocal row is then
broadcast to 128 partitions with a tiny K=32 ones-matmul, and the
normalize multiplies OT (copied to SBUF bf16 per head) by it.

Transposed-operand formulation (no on-device transposes):
  KT8[o,t]  = wk8.T @ hk8      (fp8 DoubleRow, stored fp8)
  V  [t,o]  = hkvT.T @ wvT     (bf16)
  QT8[o,s]  = wq8.T @ hq8      (fp8 DoubleRow, stored fp8)
  ST [t,s]  = KT8_h.T @ QT8_h  (fp8 operands, bf16-rate)
  ET        = exp(ST*ES) * exp(pbT)    (ACT exp over chunk pairs, DVE mult)
  OT[o,s]  += V_h.T @ ET       -> OT_sb bf16
  Zq[32h,s] = ones32.T @ ET    (4 concurrent column strips, one PSUM bank)
  rz        = recip(Zq); rz128_h = ones.T @ rz_h (K=32 broadcast matmul)
  AT        = OT_sb * rz128
  out[s,m] += AT_h.T @ woT

DMA queue split: all input loads go through the Sync HWDGE ring (pure
prefetch FIFO), all output stores through GpSimd SWDGE, so stores waiting
on compute never head-of-line-block the next block's prefetches.
"""

import math
import os

import numpy as np
import ml_dtypes

import concourse.bass as bass
import concourse.bacc as bacc
import concourse.tile as tile
from concourse import mybir
from concourse.bass_utils import run_bass_kernel_spmd

BF16 = ml_dtypes.bfloat16
FP8 = mybir.dt.np(mybir.dt.float8e4)

# Problem shapes (hardcoded per contest contract).
B, LQ, LK = 2, 2048, 2048
DM, H, DH = 2048, 16, 128
P = 128            # partitions
NCORES = 8
HPC = 4            # heads per core
OC = HPC * DH      # 512 output-proj contraction per core
DC = DM // P       # 16 d-chunks
TC = LK // P       # 16 t-chunks
SB = 4             # s-blocks per 2048
NB = LQ // SB      # 512
NPAIR = TC // 2    # 8 score-chunk pairs per block

W8SCALE = 64.0     # host pre-scale for fp8 weights
KV_SCALE = 1.0 / math.sqrt(DM)
OUT_SCALE = 1.0 / math.sqrt(H * DH)
# KT8 = K_A * k_true, QT8 = Q_A * q_true / sqrt(DH); exp undoes K_A*Q_A.
K_A = 64.0
Q_A = 512.0
K8_COPY = K_A / (W8SCALE * math.sqrt(DM))
Q8_COPY = Q_A / (W8SCALE * math.sqrt(DM) * math.sqrt(DH))
EXP_SCALE = 1.0 / (K_A * Q_A)

_PROGRAM = None          # cached compiled Bass program
_LAST_RESULTS = None     # BassKernelResults from the most recent run


def build_program():
    f32 = mybir.dt.float32
    bf16 = mybir.dt.bfloat16
    f8 = mybir.dt.float8e4
    DR = mybir.MatmulPerfMode.DoubleRow
    nc = bacc.Bacc()

    # Streamed tensors are stored block-major so every DMA slice is fully
    # contiguous (8-16KB per-partition lines -> full HBM rate).
    hq8 = nc.dram_tensor("hq8", [SB, P, DC, NB], f8, kind="ExternalInput")
    hk8 = nc.dram_tensor("hk8", [SB, P, DC, NB], f8, kind="ExternalInput")
    hkv = nc.dram_tensor("hkv", [SB, P, DC, NB], bf16, kind="ExternalInput")
    wq8 = nc.dram_tensor("wq8", [P, DC, OC], f8, kind="ExternalInput")
    wk8 = nc.dram_tensor("wk8", [P, DC, OC], f8, kind="ExternalInput")
    wvT = nc.dram_tensor("wvT", [P, DC, OC], bf16, kind="ExternalInput")
    woT = nc.dram_tensor("woT", [P, HPC, DM], bf16, kind="ExternalInput")
    pbe = nc.dram_tensor("pbe", [HPC, SB, P, TC, NB], bf16, kind="ExternalInput")
    out = nc.dram_tensor("out", [P, LQ // P, DM], f32, kind="ExternalOutput")

    Copy = mybir.ActivationFunctionType.Copy
    Exp = mybir.ActivationFunctionType.Exp
    Mult = mybir.AluOpType.mult

    with tile.TileContext(nc) as tc:
        with (
            tc.tile_pool(name="persist", bufs=1) as persist,
            tc.tile_pool(name="kv", bufs=1) as kvp,
            tc.tile_pool(name="h8s", bufs=3) as h8s,
        ):
            KT8 = kvp.tile([P, HPC, LK], f8)
            V = kvp.tile([P, TC, OC], bf16)

            def emit_hq_dma(j):
                # Allocated from the same rotating pool as the hk slices:
                # the pool WAR semaphore gates this DMA until a Kproj block
                # has consumed its buffer, so these non-startup-critical
                # loads can't steal HBM bandwidth from the critical stream.
                # (ACT HWDGE ring, so pb loads on the Sync ring can never
                # head-of-line-block the next QT projection either.)
                hq_sl = h8s.tile([P, DC, NB], f8, tag="h8", name="hq_sl")
                nc.scalar.dma_start(out=hq_sl, in_=hq8[j])
                return hq_sl

            # ---- KT / V projections (hidden_kv) ----
            with (
                tc.tile_pool(name="wkv", bufs=1) as wkvp,
                tc.tile_pool(name="hstream", bufs=2) as hs,
                tc.tile_pool(name="psA", bufs=6, space="PSUM") as psA,
            ):
                # Warmup matmuls: fill the cold-start DMA wait with junk PE
                # work so HAM unthrottles before the real stream begins.
                warm = persist.tile([P, P], bf16, name="warm")
                nc.vector.memset(warm, 0.0)
                wps = psA.tile([P, P], f32, tag="psA")
                NWARM = 168
                for i in range(NWARM):
                    nc.tensor.matmul(
                        wps, lhsT=warm, rhs=warm,
                        start=(i == 0), stop=(i == NWARM - 1),
                    )

                # K projections first: only 2MB of fp8 (wk8 + first slice) is
                # startup-critical; V's bf16 loads trail behind on the ring.
                wk_sb = wkvp.tile([P, DC, OC], f8)
                nc.sync.dma_start(out=wk_sb, in_=wk8[:])
                k_sl0 = h8s.tile([P, DC, NB], f8, tag="h8")
                nc.sync.dma_start(out=k_sl0, in_=hk8[0])
                ones_sb = persist.tile([P, P], bf16)
                nc.vector.memset(ones_sb, 1.0)
                woT_sb = persist.tile([P, HPC, DM], bf16)
                # woT is the only ungated early load on the ACT ring; it
                # streams while the Sync ring feeds the K projections.
                nc.scalar.dma_start(out=woT_sb, in_=woT[:])
                hq_tiles = []

                for j in range(SB):
                    if j == 0:
                        k_sl = k_sl0
                    else:
                        k_sl = h8s.tile([P, DC, NB], f8, tag="h8")
                        nc.sync.dma_start(out=k_sl, in_=hk8[j])
                    for h in range(HPC):
                        ps = psA.tile([P, NB], f32, tag="psA")
                        for d in range(0, DC, 2):
                            nc.tensor.matmul(
                                ps,
                                lhsT=wk_sb[:, d:d + 2, h * P:(h + 1) * P],
                                rhs=k_sl[:, d:d + 2, :],
                                start=(d == 0),
                                stop=(d == DC - 2),
                                perf_mode=DR,
                            )
                        nc.scalar.activation(
                            KT8[:, h, j * NB:(j + 1) * NB], ps, Copy, scale=K8_COPY
                        )
                # hq / wq ride the h8s pool rotation: each DMA is WAR-gated
                # on a consumed hk buffer (Kproj j1/j2/j3 done), keeping the
                # early HBM bandwidth for the startup-critical stream.
                hq_tiles += [emit_hq_dma(0), emit_hq_dma(1)]
                wq_sb = h8s.tile([P, DC, OC], f8, tag="h8", name="wq_sb")
                nc.scalar.dma_start(out=wq_sb, in_=wq8[:])

                wv_sb = wkvp.tile([P, DC, OC], bf16)
                nc.sync.dma_start(out=wv_sb, in_=wvT[:])
                for j in range(SB):
                    h_sl = hs.tile([P, DC, NB], bf16, tag="h")
                    nc.sync.dma_start(out=h_sl, in_=hkv[j])
                    for t4 in range(4):
                        ps = psA.tile([P, NB], f32, tag="psA")
                        for d in range(DC):
                            nc.tensor.matmul(
                                ps,
                                lhsT=h_sl[:, d, t4 * P:(t4 + 1) * P],
                                rhs=wv_sb[:, d, :],
                                start=(d == 0),
                                stop=(d == DC - 1),
                            )
                        nc.scalar.activation(
                            V[:, j * 4 + t4, :], ps, Copy, scale=KV_SCALE
                        )

            # ---- fused main loop over s-blocks ----
            with (
                tc.tile_pool(name="qt", bufs=2) as qtp,
                tc.tile_pool(name="at", bufs=1) as atp,
                tc.tile_pool(name="ot", bufs=2) as otp,
                tc.tile_pool(name="pb", bufs=3) as pbp,
                tc.tile_pool(name="es", bufs=2) as esp,
                tc.tile_pool(name="E", bufs=4) as Ep,
                tc.tile_pool(name="rz", bufs=1) as rzp,
                tc.tile_pool(name="cst", bufs=2) as csp,
                tc.tile_pool(name="psS", bufs=2, space="PSUM") as psS,
                tc.tile_pool(name="psOP", bufs=1, space="PSUM") as psOP,
                tc.tile_pool(name="psO", bufs=1, space="PSUM") as psO,
                tc.tile_pool(name="psZ", bufs=1, space="PSUM") as psZ,
            ):
                # PE work-stealing queue: single-matmul thunks of dependency-
                # free deferred work (previous block's out-projection, next
                # block's QT projection) that are woven between the S/AV
                # matmuls so PE never idles while ACT works through the exps.
                stolen = []

                def steal(n):
                    for _ in range(min(n, len(stolen))):
                        stolen.pop(0)()

                def queue_qt_proj(hq_sl):
                    """Queue the next block's QT projection; returns the
                    (not-yet-written) fp8 QT tile."""
                    QT8n = qtp.tile([P, HPC, NB], f8, tag="qt", name="QT8n")
                    for hp in range(HPC // 2):
                        ps = psOP.tile([P, 2 * NB], f32, tag="op", name="psq")
                        for h2 in range(2):
                            h = 2 * hp + h2
                            for d in range(0, DC, 2):
                                def mm(h=h, d=d, ps=ps, h2=h2):
                                    nc.tensor.matmul(
                                        ps[:, h2 * NB:(h2 + 1) * NB],
                                        lhsT=wq_sb[:, d:d + 2, h * P:(h + 1) * P],
                                        rhs=hq_sl[:, d:d + 2, :],
                                        start=(d == 0),
                                        stop=(d == DC - 2),
                                        perf_mode=DR,
                                        skip_group_check=True,
                                    )
                                stolen.append(mm)
                        prev = stolen.pop()

                        def last_mm(prev=prev, hp=hp, ps=ps):
                            prev()
                            nc.vector.tensor_scalar_mul(
                                QT8n[:, 2 * hp:2 * hp + 2, :],
                                ps.rearrange("p (c n) -> p c n", c=2),
                                Q8_COPY,
                            )
                        stolen.append(last_mm)
                    return QT8n

                def queue_outproj(ATj, j):
                    """Queue block j's out-projection (row-parallel partial)."""
                    for sc4 in range(NB // P):
                        sc = j * (NB // P) + sc4
                        for mbp in range(DM // NB // 2):
                            ps = psOP.tile([P, 2 * NB], f32, tag="op", name="psop")
                            for mb2 in range(2):
                                mb = 2 * mbp + mb2
                                for oc in range(HPC):
                                    def mm(ps=ps, mb2=mb2, mb=mb, oc=oc,
                                           ATj=ATj, sc4=sc4):
                                        nc.tensor.matmul(
                                            ps[:, mb2 * NB:(mb2 + 1) * NB],
                                            lhsT=ATj[:, oc, sc4 * P:(sc4 + 1) * P],
                                            rhs=woT_sb[:, oc, mb * NB:(mb + 1) * NB],
                                            start=(oc == 0),
                                            stop=(oc == HPC - 1),
                                            skip_group_check=True,
                                        )
                                    stolen.append(mm)
                            def store(ps=ps, sc=sc, mbp=mbp):
                                cst = csp.tile([P, 2 * NB], f32, tag="cs")
                                nc.vector.tensor_scalar_mul(cst, ps, OUT_SCALE)
                                nc.gpsimd.dma_start(
                                    out=out[:, sc, mbp * 2 * NB:(mbp + 1) * 2 * NB],
                                    in_=cst,
                                )
                            prev = stolen.pop()
                            def last_mm(prev=prev, store=store):
                                prev()
                                store()
                            stolen.append(last_mm)

                # Rolling position-bias prefetch, 2 blocks deep.
                blocks = [(j, h) for j in range(SB) for h in range(HPC)]

                def emit_pb_dma(j, h):
                    pb_sl = pbp.tile([P, TC, NB], bf16, tag="pb", name="pb_sl")
                    nc.sync.dma_start(out=pb_sl, in_=pbe[h, j])
                    return pb_sl

                pb_tiles = {bl: emit_pb_dma(*bl) for bl in blocks[:2]}

                def emit_chain_piece(hh, rz_bf, OT_prev, AT_prev):
                    """One head's reciprocal broadcast + fused normalize.
                    Woven between S pairs so the bcast's wait on the DVE
                    chain hides behind score streaming."""
                    rz_ps = psZ.tile([P, NB], f32, tag="psZ")
                    nc.tensor.matmul(
                        rz_ps,
                        lhsT=ones_sb[32 * hh:32 * (hh + 1), :],
                        rhs=rz_bf[32 * hh:32 * (hh + 1), :],
                        start=True,
                        stop=True,
                        tile_position=(32 * hh, 0),
                    )
                    nc.vector.scalar_tensor_tensor(
                        AT_prev[:, hh, :], OT_prev[:, hh, :], 1.0 / 32.0,
                        rz_ps, Mult, Mult,
                    )

                # j=0's QT projection runs immediately (nothing to overlap).
                QT8j = queue_qt_proj(hq_tiles.pop(0))
                steal(len(stolen))
                hq_next = hq_tiles.pop(0)
                chain_state = None   # (Zq_ps, OT_sb) of the previous block
                for j in range(SB):
                    # QT for j+1 is queued FIRST: the psOP pool rotation
                    # makes queue order execution order, and j+1's score
                    # matmuls are blocked until its QT completes — it must
                    # clear early in the stream, not at the boundary.
                    if j < SB - 1:
                        QT8next = queue_qt_proj(hq_next)
                        if j < SB - 2:
                            hq_next = emit_hq_dma(j + 2)
                        elif j == SB - 2:
                            hq_next = None

                    OT_sb = otp.tile([P, HPC, NB], bf16, tag="ot")
                    E_tiles = []
                    for h in range(HPC):
                        pb_sl = pb_tiles.pop((j, h))
                        ahead = blocks.index((j, h)) + 2
                        if ahead < len(blocks):
                            pb_tiles[blocks[ahead]] = emit_pb_dma(*blocks[ahead])
                        E_sl = Ep.tile([P, TC, NB], bf16, tag="E")
                        E_tiles.append(E_sl)
                        O_ps = psO.tile([P, NB], f32, tag="psO")

                        def av(t):
                            nc.tensor.matmul(
                                O_ps,
                                lhsT=V[:, t, h * DH:(h + 1) * DH],
                                rhs=E_sl[:, t, :],
                                start=(t == 0),
                                stop=(t == TC - 1),
                                skip_group_check=True,
                            )

                        for p in range(NPAIR):
                            if h == 0 and chain_state is not None and p == 0:
                                Zq_prev, OT_prev = chain_state
                                rz_f32 = rzp.tile([P, NB], f32, tag="rz")
                                nc.vector.reciprocal_approx_fast(rz_f32, Zq_prev)
                                rz_bf = rzp.tile([P, NB], bf16, tag="rzc")
                                nc.vector.tensor_copy(rz_bf, rz_f32)
                                AT_prev = atp.tile(
                                    [P, HPC, NB], bf16, tag="at", name="AT_prev"
                                )
                            S_ps = psS.tile([P, 2 * NB], f32, tag="big")
                            for q in range(2):
                                nc.tensor.matmul(
                                    S_ps[:, q * NB:(q + 1) * NB],
                                    lhsT=KT8[:, h, (2 * p + q) * P:(2 * p + q + 1) * P],
                                    rhs=QT8j[:, h, :],
                                    start=True,
                                    stop=True,
                                    skip_group_check=True,
                                )
                            if h == 0 and chain_state is not None and p < HPC:
                                emit_chain_piece(p, rz_bf, OT_prev, AT_prev)
                                if p == HPC - 1:
                                    queue_outproj(AT_prev, j - 1)
                                    chain_state = None
                            steal(1 if p >= 4 else 2)
                            eS = esp.tile([P, 2 * NB], bf16, tag="es")
                            nc.scalar.activation(eS, S_ps, Exp, scale=EXP_SCALE)
                            nc.vector.tensor_tensor(
                                E_sl[:, 2 * p:2 * p + 2, :],
                                eS.rearrange("p (c n) -> p c n", c=2),
                                pb_sl[:, 2 * p:2 * p + 2, :],
                                Mult,
                            )
                            if p >= 2:
                                av(2 * p - 4)
                                av(2 * p - 3)
                        for t in range(TC - 4, TC):
                            av(t)
                            steal(1)
                        # Free the PSUM bank for the next head; the end-of-
                        # block normalize reads the bf16 SBUF copy instead.
                        nc.vector.tensor_copy(OT_sb[:, h, :], O_ps)

                    # Softmax denominators for all 4 heads as concurrent
                    # 32-wide column strips: one matmul stream's worth of PE
                    # time instead of four.
                    Zq_ps = psZ.tile([P, NB], f32, tag="psZ")
                    for t in range(TC):
                        for h in range(HPC):
                            nc.tensor.matmul(
                                Zq_ps[32 * h:32 * (h + 1), :],
                                lhsT=ones_sb[:, 0:32],
                                rhs=E_tiles[h][:, t, :],
                                start=(t == 0),
                                stop=(t == TC - 1),
                                skip_group_check=True,
                                tile_position=(0, 32 * h),
                            )

                    # Drain any leftover deferred matmuls.
                    steal(len(stolen))
                    chain_state = (Zq_ps, OT_sb)
                    if j < SB - 1:
                        QT8j = QT8next

                # Epilogue: the last block's normalize chain, then its
                # out-projection, double-buffered through the psS banks so
                # the DVE scale of tile i overlaps the matmuls of tile i+1.
                Zq_prev, OT_prev = chain_state
                rz_f32 = rzp.tile([P, NB], f32, tag="rz")
                nc.vector.reciprocal_approx_fast(rz_f32, Zq_prev)
                rz_bf = rzp.tile([P, NB], bf16, tag="rzc")
                nc.vector.tensor_copy(rz_bf, rz_f32)
                AT_prev = atp.tile([P, HPC, NB], bf16, tag="at", name="AT_prev")
                for hh in range(HPC):
                    emit_chain_piece(hh, rz_bf, OT_prev, AT_prev)

                # Final block's out-projection has nothing left to hide
                # behind; emit it directly, double-buffered through the psS
                # banks (free by now) so the DVE scale of tile i overlaps
                # the matmuls of tile i+1.
                for sc4 in range(NB // P):
                    sc = (SB - 1) * (NB // P) + sc4
                    for mbp in range(DM // NB // 2):
                        ps = psS.tile([P, 2 * NB], f32, tag="big")
                        for mb2 in range(2):
                            mb = 2 * mbp + mb2
                            for oc in range(HPC):
                                nc.tensor.matmul(
                                    ps[:, mb2 * NB:(mb2 + 1) * NB],
                                    lhsT=AT_prev[:, oc, sc4 * P:(sc4 + 1) * P],
                                    rhs=woT_sb[:, oc, mb * NB:(mb + 1) * NB],
                                    start=(oc == 0),
                                    stop=(oc == HPC - 1),
                                    skip_group_check=True,
                                )
                        cst = csp.tile([P, 2 * NB], f32, tag="cs")
                        nc.vector.tensor_scalar_mul(cst, ps, OUT_SCALE)
                        nc.gpsimd.dma_start(
                            out=out[:, sc, mbp * 2 * NB:(mbp + 1) * 2 * NB],
                            in_=cst,
                        )

    nc.compile()
    return nc


def _get_program():
    global _PROGRAM
    if _PROGRAM is None:
        _PROGRAM = build_program()
    return _PROGRAM


def make_in_maps(hidden_q, hidden_kv, attention_mask, position_bias, wq, wk, wv, wo):
    """Host-side shard + transpose + cast for all 8 cores."""
    f32 = np.float32

    def dxp(x):  # [n, (dc p)] -> [p, dc, n]  (transpose with d on partitions)
        n = x.shape[0]
        return np.ascontiguousarray(x.reshape(n, DC, P).transpose(2, 1, 0))

    def blocked(t):  # [p, dc, n] -> [SB, p, dc, NB]  (contiguous DMA slices)
        return np.ascontiguousarray(
            t.reshape(P, DC, SB, NB).transpose(2, 0, 1, 3)
        )

    hq8_b = [blocked(dxp(np.asarray(hidden_q[b], f32))).astype(FP8) for b in range(B)]
    hkv_t = [blocked(dxp(np.asarray(hidden_kv[b], f32))) for b in range(B)]
    hk8_b = [t.astype(FP8) for t in hkv_t]
    hkv_b = [t.astype(BF16) for t in hkv_t]

    mask = np.asarray(attention_mask)
    mask_all_ones = bool(mask.all())

    w_by_hg = []
    for hg in range(HPC):
        rows = slice(hg * OC, (hg + 1) * OC)
        wq8 = (dxp(np.asarray(wq[rows], f32)) * W8SCALE).astype(FP8)
        wk8 = (dxp(np.asarray(wk[rows], f32)) * W8SCALE).astype(FP8)
        wvT = dxp(np.asarray(wv[rows], f32)).astype(BF16)
        woT = np.ascontiguousarray(
            np.asarray(wo[:, rows], f32).reshape(DM, HPC, P).transpose(2, 1, 0)
        ).astype(BF16)
        w_by_hg.append((wq8, wk8, wvT, woT))

    in_maps = []
    for core in range(NCORES):
        b, hg = divmod(core, HPC)
        pb_sel = np.asarray(position_bias[hg * HPC:(hg + 1) * HPC], f32)
        pbT = pb_sel.reshape(HPC, LQ, TC, P).transpose(0, 3, 2, 1)  # [h,p,tc,s]
        pbe = np.exp(pbT, dtype=f32)
        if not mask_all_ones:
            # mask folded multiplicatively into exp(pb): zeroed keys drop out
            # of both the numerator and the softmax denominator, matching
            # where(mask, score, -inf) + where(mask, probs, 0).
            mT = mask[b].T.reshape(TC, P, LQ).transpose(1, 0, 2)
            pbe = pbe * mT[None].astype(f32)
        # block-major on s: [h, p, tc, s] -> [h, SB, p, tc, NB]
        pbe = np.ascontiguousarray(
            pbe.reshape(HPC, P, TC, SB, NB).transpose(0, 3, 1, 2, 4)
        )
        wq8, wk8, wvT, woT = w_by_hg[hg]
        in_maps.append(
            {
                "hq8": hq8_b[b],
                "hk8": hk8_b[b],
                "hkv": hkv_b[b],
                "wq8": wq8,
                "wk8": wk8,
                "wvT": wvT,
                "woT": woT,
                "pbe": pbe.astype(BF16),
            }
        )
    return in_maps


def gather_output(results):
    """Sum the 4 row-parallel partials per batch; un-permute to [B, LQ, DM]."""
    out = np.zeros((B, LQ, DM), np.float32)
    for core in range(NCORES):
        b = core // HPC
        part = results[core]["out"]  # [P, LQ//P, DM]
        out[b] += part.transpose(1, 0, 2).reshape(LQ, DM)
    return out


def kernel(hidden_q, hidden_kv, attention_mask, position_bias, wq, wk, wv, wo):
    global _LAST_RESULTS
    nc = _get_program()
    in_maps = make_in_maps(
        hidden_q, hidden_kv, attention_mask, position_bias, wq, wk, wv, wo
    )
    trace = os.environ.get("KERNEL_TRACE", "0") == "1"
    res = run_bass_kernel_spmd(
        nc,
        in_maps,
        core_ids=list(range(NCORES)),
        trace=trace,
        trace_cores=[0] if trace else None,
    )
    _LAST_RESULTS = res
    return gather_output(res.results)
